# revision 1
# baseline (speedup 1.0000x reference)
"""AttentionBlock (GroupNorm(32) + 1-head self-attention + proj + residual) on 8 trn2 cores.

Data-parallel over batch: each of the 8 NeuronCores processes 2 of the 16 images.

Algebraic fusion (valid because the reference's q/k biases are zero):
  scores: s_ij = q_i.k_j = xn_i^T (Wq^T Wk) xn_j. Precompute M = Wq^T Wk
          host-side, compute m = M^T xn on device (ONE projection instead of
          q and k), then s_ij = m_i . xn_j with xn as the stationary operand.
  output: proj(attn@V) = sum_j p_j (Wp Wv xn_j). Precompute W' = Wp Wv, so
          u = W' xn replaces v and the separate projection matmul vanishes.
This removes ~25% of the matmul work and two fp8 quantization stages
(attention output and Wp), improving accuracy.

All large matmuls run in fp8e4 with perf_mode=DoubleRow (K=256 per pass).
GroupNorm statistics stay in float32r for accuracy. Scaling scheme (fp8e4 has
3 mantissa bits, TRN max +-240, min normal 2^-6, so operands sit near unit
scale): M and W' are scaled by 16 host-side (entries ~N(0,1/512) would be
subnormal); the softmax scale c^-0.5 and the 1/16 fold into the exp
activation's scale; a -2.0 bias keeps exp outputs <= ~40 (cancels in
softmax). u carries 16x, which cancels against the all-16s matmul used for
the softmax denominator (recip of 16*l).

Layout (per image, c=512 channels, n=1024 positions):
  - x in (channel, position) f32r [128, 4, 1024]; xn/m in fp8 [128, 4, 1024]
    (partition=c%128, dim1=c//128; DoubleRow consumes k-tile pairs
    [:, 2u:2u+2, :]).
  - u computed directly transposed (position, channel) fp8 [128, 8, 512];
    exp(scores) transposed fp8 [128, 8, 1024]: the attention contraction
    over positions j pairs j-tiles the same way.
  - softmax column sums via an all-16s fp8 DoubleRow matmul, which also
    broadcasts across partitions for free; 1/(16 l) via
    reciprocal_approx_fast.
  - GroupNorm stats via f32r indicator matmuls (indicator pre-scaled by
    1/(16*1024)); rstd via Newton rsqrt on DVE; ScalarE only ever needs one
    ACT table set (exp/copy/identity).
  - A post-emission pass drops InstLdweights that reload identical weights
    (DoubleRow turns off FWL and serializes the 256-column load otherwise).
  - DMAs are batched; per-phase emission order software-pipelines the two
    images (image 2's GroupNorm chain runs under image 1's attention).
"""

import ml_dtypes
import numpy as np

import concourse.bacc as bacc
import concourse.tile as tile
import concourse.mybir as mybir
from concourse.bass_utils import run_bass_kernel_spmd

F32 = mybir.dt.float32
F32R = mybir.dt.float32r
F8 = mybir.dt.float8e4
I32 = mybir.dt.int32
AF = mybir.ActivationFunctionType
ALU = mybir.AluOpType
AX = mybir.AxisListType
DR = mybir.MatmulPerfMode.DoubleRow

B, C, H, W = 16, 512, 32, 32
N = H * W                 # 1024 positions
NCORES = 8
BPC = B // NCORES         # 2 images per core
G = 32                    # groupnorm groups
GS = C // G               # 16 channels per group
CT = C // 128             # 4 channel tiles
NT = N // 128             # 8 position tiles
NH = N // 512             # 2 free-dim halves
EPS = 1e-5
SCALE = float(C) ** -0.5  # single head, head_dim = C
WSC = 16.0                # host-side weight scale (power of 2, exact in fp8)
EXP_SCALE = SCALE / WSC   # m carries 16x; exp undoes it + softmax scale
EXP_BIAS = -2.0
MAGIC = 0x5F3759DF        # Newton-rsqrt seed constant

_cache: dict = {}


def _dedup_ldweights(nc):
    """Drop InstLdweights that reload the identical weights AP.

    The DoubleRow split pass emits one InstLdweights per matmul even when
    consecutive matmuls use the same stationary operand; the PE array
    retains loaded weights. Self-loading f32/f32r matmuls clobber the
    array, so they reset the tracking. Loads carrying semaphore
    waits/updates are kept.
    """
    ndrop = 0
    for f in nc.m.functions:
        for blk in f.blocks:
            insts = list(blk.instructions)
            drop = []
            last_key = None
            for idx, inst in enumerate(insts):
                nm = type(inst).__name__
                if nm == "InstLdweights":
                    si = inst.sync_info
                    has_sync = si is not None and (
                        len(si.on_wait) > 0 or len(si.on_update) > 0
                    )
                    key = str(inst.ins[0])
                    if key == last_key and not has_sync:
                        drop.append(idx)
                    else:
                        last_key = key
                elif nm == "InstMatmult":
                    if inst.perf_mode is None:
                        last_key = None
            for idx in reversed(drop):
                del blk.instructions[idx]
            ndrop += len(drop)
    return ndrop


def _build(loop_iters: int = 0):
    nc = bacc.Bacc("TRN2", target_bir_lowering=False, num_devices=NCORES)

    # f32r DRAM tensors so nc.sync DMA needs no cast (numpy side is float32).
    x_d = nc.dram_tensor("x", [BPC, C, N], F32R, kind="ExternalInput")
    wm_d = nc.dram_tensor("wm", [C, C], F8, kind="ExternalInput")  # 16*(Wq^T Wk), m = wm^T xn
    wu_d = nc.dram_tensor("wu", [C, C], F8, kind="ExternalInput")  # 16*(Wp Wv)^T, u = wu^T xn
    ind_d = nc.dram_tensor("ind", [C, G], F32R, kind="ExternalInput")  # (1/(16*1024)) iff c//16==g
    bind_d = nc.dram_tensor("bind", [G, C], F32R, kind="ExternalInput")  # 0/1 indicator.T
    ones_d = nc.dram_tensor("onesm", [128, 256], F8, kind="ExternalInput")  # all 16.0
    # consts: [gnsc | gnbi], each (128, CT)
    consts_d = nc.dram_tensor("consts", [128, 2 * CT], F32, kind="ExternalInput")
    out_d = nc.dram_tensor("out", [BPC, C, N], F32, kind="ExternalOutput")

    with tile.TileContext(nc) as tc:
        with (
            tc.tile_pool(name="wpool", bufs=1) as wp_,
            tc.tile_pool(name="xpool", bufs=2) as xpool,
            tc.tile_pool(name="xnpool", bufs=2) as xnpool,
            tc.tile_pool(name="sqpool", bufs=1) as sqpool,
            tc.tile_pool(name="mpool", bufs=2) as mpool,
            tc.tile_pool(name="upool", bufs=2) as upool,
            tc.tile_pool(name="epool", bufs=2) as epool,
            tc.tile_pool(name="fpool", bufs=1) as fpool,
            tc.tile_pool(name="rpool", bufs=2) as rpool,
            tc.tile_pool(name="spool", bufs=2) as spool,
            tc.tile_pool(name="psA", bufs=6, space="PSUM") as psA,
            tc.tile_pool(name="psB", bufs=2, space="PSUM") as psB,
        ):
            # ---- persistent constants / weights (batched single DMAs) ----
            wm_all = wp_.tile([128, CT, C], F8, tag="wm", name="wm")
            wu_all = wp_.tile([128, CT, C], F8, tag="wu", name="wu")
            ind_all = wp_.tile([128, CT, G], F32R, tag="ind", name="ind")
            bind_all = wp_.tile([G, CT, 128], F32R, tag="bind", name="bind")
            ones_sb = wp_.tile([128, 2, 128], F8, tag="ones", name="ones")
            consts_sb = wp_.tile([128, 2 * CT], F32, tag="consts", name="consts")
            magic_sb = wp_.tile([128, 1], I32, tag="magic", name="magic")
            nc.vector.memset(magic_sb, MAGIC)
            exp_sc = wp_.tile([128, 1], F32, tag="expsc", name="expsc")
            nc.vector.memset(exp_sc, EXP_SCALE)
            exp_bi = wp_.tile([128, 1], F32, tag="expbi", name="expbi")
            nc.vector.memset(exp_bi, EXP_BIAS)
            gnsc_sb = consts_sb[:, 0 * CT:1 * CT]
            gnbi_sb = consts_sb[:, 1 * CT:2 * CT]

            def part(dram2d):
                # (T*128, F) -> [128, T, F]
                return dram2d.rearrange("(t p) f -> p t f", p=128)

            # issue order matters: the first stats matmul needs only
            # ind + the first half of x tile 0 -- land those first
            x0 = xpool.tile([128, CT, N], F32R, tag="x", name="x")
            nc.sync.dma_start(out=x0[:, 0, 0:512], in_=x_d[0, 0:128, 0:512])
            nc.sync.dma_start(out=ind_all, in_=part(ind_d[:, :]))
            nc.sync.dma_start(out=x0[:, 0, 512:1024], in_=x_d[0, 0:128, 512:1024])
            for t in range(1, CT):
                nc.sync.dma_start(
                    out=x0[:, t, :], in_=x_d[0, 128 * t:128 * (t + 1), :]
                )
            nc.sync.dma_start(out=consts_sb, in_=consts_d[:, :])
            nc.sync.dma_start(
                out=bind_all, in_=bind_d.rearrange("g (t p) -> g t p", p=128)
            )
            nc.sync.dma_start(out=wm_all, in_=part(wm_d[:, :]))
            nc.sync.dma_start(out=wu_all, in_=part(wu_d[:, :]))
            nc.sync.dma_start(
                out=ones_sb, in_=ones_d.rearrange("p (s f) -> p s f", s=2)
            )

            halves = [slice(0, 512), slice(512, 1024)]

            def emit_gn(img, x_pre=None):
                """Load x, GroupNorm stats + chain, xn apply (fp8 out).
                Returns state dict for later phases."""
                if x_pre is not None:
                    x_all = x_pre
                else:
                    x_all = xpool.tile([128, CT, N], F32R, tag="x", name="x")
                    for t in range(CT):
                        nc.sync.dma_start(
                            out=x_all[:, t, :],
                            in_=x_d[img, 128 * t:128 * (t + 1), :],
                        )
                xt = [x_all[:, t, :] for t in range(CT)]

                # x^2
                xsq_all = sqpool.tile([128, CT, N], F32R, tag="sq", name="sq")
                xf = x_all.bitcast(F32)
                for t in range(CT):
                    nc.vector.tensor_mul(xsq_all[:, t, :], xf[:, t, :], xf[:, t, :])

                # group means via (1/(16*1024))-scaled indicator matmuls
                sum_ps = psB.tile([G, 512], F32, tag="psB", name="psB")
                sum_ps2 = psB.tile([G, 512], F32, tag="psB", name="psB")
                sq_ps = psB.tile([G, 512], F32, tag="psB", name="psB")
                sq_ps2 = psB.tile([G, 512], F32, tag="psB", name="psB")
                for t in range(CT):
                    for h, ps in ((0, sum_ps), (1, sum_ps2)):
                        nc.tensor.matmul(
                            ps[:, :], ind_all[:, t, :], xt[t][:, halves[h]],
                            start=(t == 0), stop=(t == CT - 1),
                        )
                for t in range(CT):
                    for h, ps in ((0, sq_ps), (1, sq_ps2)):
                        nc.tensor.matmul(
                            ps[:, :], ind_all[:, t, :], xsq_all[:, t, halves[h]],
                            start=(t == 0), stop=(t == CT - 1),
                        )

                # reduce to [mean_h0, mean_h1, e2_h0, e2_h1] (already scaled)
                st4 = spool.tile([G, 4], F32, tag="st4", name="st4")
                recipbc = rpool.tile([128, N], F32, tag="rbc", name="rbc")
                nc.vector.reduce_sum(out=st4[:, 0:1], in_=sum_ps[:, :], axis=AX.X)
                nc.vector.reduce_sum(out=st4[:, 1:2], in_=sum_ps2[:, :], axis=AX.X)
                # recipbc is fully overwritten much later; use as throwaway ACT
                # output (we only want accum_out).
                nc.scalar.activation(out=recipbc[0:G, 0:512], in_=sq_ps[:, :],
                                     func=AF.Copy, accum_out=st4[:, 2:3])
                nc.scalar.activation(out=recipbc[0:G, 512:1024], in_=sq_ps2[:, :],
                                     func=AF.Copy, accum_out=st4[:, 3:4])
                # e12 = [mean, E[x^2]]
                e12 = spool.tile([G, 2], F32, tag="e12", name="e12")
                st4_v = st4.rearrange("g (s h) -> g s h", h=2)
                nc.vector.tensor_add(e12[:, :], st4_v[:, :, 0], st4_v[:, :, 1])
                # vpe = E2 + eps - mean^2
                vpe = spool.tile([G, 1], F32, tag="vpe", name="vpe")
                msq = spool.tile([G, 1], F32, tag="msq", name="msq")
                nc.vector.tensor_mul(msq[:, :], e12[:, 0:1], e12[:, 0:1])
                nc.vector.scalar_tensor_tensor(
                    out=vpe[:, :], in0=e12[:, 1:2], scalar=EPS, in1=msq[:, :],
                    op0=ALU.add, op1=ALU.subtract,
                )
                # rstd = 1/sqrt(vpe): bit-trick seed + 2 Newton iterations
                sh_t = spool.tile([G, 1], I32, tag="sh", name="sh")
                nc.vector.tensor_scalar(
                    out=sh_t[:, :], in0=vpe.bitcast(I32)[:, :], scalar1=1,
                    scalar2=None, op0=ALU.logical_shift_right,
                )
                seed = spool.tile([G, 1], I32, tag="seed", name="seed")
                nc.vector.scalar_tensor_tensor(
                    out=seed[:, :], in0=magic_sb[:G, :], scalar=0, in1=sh_t[:, :],
                    op0=ALU.bypass, op1=ALU.subtract,
                )
                y = seed.bitcast(F32)
                for it in range(2):
                    t1 = spool.tile([G, 1], F32, tag=f"nr{it}", name=f"nr{it}")
                    nc.vector.tensor_mul(t1[:, :], y[:, :], y[:, :])
                    nc.vector.tensor_mul(t1[:, :], t1[:, :], vpe[:, :])
                    nc.vector.tensor_scalar(
                        out=t1[:, :], in0=t1[:, :], scalar1=-0.5, scalar2=1.5,
                        op0=ALU.mult, op1=ALU.add,
                    )
                    y2 = spool.tile([G, 1], F32, tag=f"y{it}", name=f"y{it}")
                    nc.vector.tensor_mul(y2[:, :], y[:, :], t1[:, :])
                    y = y2
                # stats2 = [rstd, mean] (f32r for the broadcast matmul)
                stats2 = spool.tile([G, 2], F32R, tag="st2", name="st2")
                nc.vector.tensor_copy(stats2[:, 0:1], y[:, :])
                nc.vector.tensor_copy(stats2[:, 1:2], e12[:, 0:1])

                # broadcast to channels; a = gnsc*rstd, b = gnbi - mean*a
                bc_ps = psB.tile([128, 2 * CT], F32, tag="psB", name="psB")
                for t in range(CT):
                    nc.tensor.matmul(
                        bc_ps[:, 2 * t:2 * t + 2], bind_all[:, t, :], stats2[:, :],
                        start=True, stop=True,
                    )
                bc_sb = spool.tile([128, 2 * CT], F32, tag="bc", name="bc")
                nc.vector.tensor_copy(bc_sb[:, :], bc_ps[:, :])
                bc_v = bc_sb.rearrange("p (t s) -> p t s", s=2)
                a_all = spool.tile([128, CT], F32, tag="aall", name="aall")
                b_all = spool.tile([128, CT], F32, tag="ball", name="ball")
                nc.vector.tensor_mul(a_all[:, :], gnsc_sb, bc_v[:, :, 0])
                nc.vector.scalar_tensor_tensor(
                    out=b_all[:, :], in0=bc_v[:, :, 1], scalar=-1.0, in1=a_all[:, :],
                    op0=ALU.mult, op1=ALU.mult,
                )
                nc.vector.tensor_add(b_all[:, :], b_all[:, :], gnbi_sb)

                # xn = a*x + b (ACT; Identity is in every table set), fp8 out
                xn_all = xnpool.tile([128, CT, N], F8, tag="xn", name="xn")
                for t in range(CT):
                    nc.scalar.activation(
                        out=xn_all[:, t, :], in_=xt[t].bitcast(F32),
                        func=AF.Identity,
                        scale=a_all[:, t:t + 1], bias=b_all[:, t:t + 1],
                    )
                return {"xt": xt, "xn": xn_all, "recipbc": recipbc, "img": img}

            def emit_mu(s):
                """m = M^T xn (scores left factor) and uT = (W' xn)^T."""
                xn = s["xn"]
                m_all = mpool.tile([128, CT, N], F8, tag="m", name="m")
                for d in range(CT):
                    ds_ = slice(128 * d, 128 * (d + 1))
                    # per-half PSUM tiles (1 bank each): finer rotation lets
                    # PE run ahead of the ACT/DVE drains
                    mps = [psA.tile([128, 512], F32, tag="psA", name="psA")
                           for _ in range(NH)]
                    for u in range(CT // 2):
                        us = slice(2 * u, 2 * u + 2)
                        for h in range(NH):
                            nc.tensor.matmul(
                                mps[h][:, :], wm_all[:, us, ds_],
                                xn[:, us, halves[h]],
                                start=(u == 0), stop=(u == CT // 2 - 1),
                                perf_mode=DR,
                            )
                    for h in range(NH):
                        nc.scalar.copy(out=m_all[:, d, halves[h]],
                                       in_=mps[h][:, :])
                # uT[n, o]: lhsT = xn[:, n-slice], rhs = wu
                uT_all = upool.tile([128, NT, C], F8, tag="u", name="u")
                for n in range(NT):
                    ns = slice(128 * n, 128 * (n + 1))
                    ups = psB.tile([128, 512], F32, tag="psB", name="psB")
                    for u in range(CT // 2):
                        us = slice(2 * u, 2 * u + 2)
                        nc.tensor.matmul(
                            ups[:, :], xn[:, us, ns], wu_all[:, us, :],
                            start=(u == 0), stop=(u == CT // 2 - 1),
                            perf_mode=DR,
                        )
                    nc.vector.tensor_copy(uT_all[:, n, :], ups[:, :])
                s["m"], s["uT"] = m_all, uT_all

            def emit_att(s):
                m, uT, xn, recipbc = s["m"], s["uT"], s["xn"], s["recipbc"]
                xt, img = s["xt"], s["img"]
                expT = epool.tile([128, NT, N], F8, tag="e", name="e")
                for j in range(NT):
                    js = slice(128 * j, 128 * (j + 1))
                    sps = [psA.tile([128, 512], F32, tag="psA", name="psA")
                           for _ in range(NH)]
                    for u in range(CT // 2):
                        us = slice(2 * u, 2 * u + 2)
                        for h in range(NH):
                            nc.tensor.matmul(
                                sps[h][:, :], xn[:, us, js],
                                m[:, us, halves[h]],
                                start=(u == 0), stop=(u == CT // 2 - 1),
                                perf_mode=DR,
                            )
                    for h in range(NH):
                        nc.scalar.activation(out=expT[:, j, halves[h]],
                                             in_=sps[h][:, :],
                                             func=AF.Exp, scale=exp_sc[:, 0:1],
                                             bias=exp_bi[:, 0:1])

                # 16*l[i] = sum_j 16*exp, broadcast across partitions (all-16s
                # matmul); recipbc = 1/(16 l) cancels u's 16x for free
                lps = [psA.tile([128, 512], F32, tag="psA", name="psA")
                       for _ in range(NH)]
                for u in range(NT // 2):
                    us = slice(2 * u, 2 * u + 2)
                    for h in range(NH):
                        nc.tensor.matmul(
                            lps[h][:, :], ones_sb[:, :, :],
                            expT[:, us, halves[h]],
                            start=(u == 0), stop=(u == NT // 2 - 1),
                            perf_mode=DR,
                        )
                for h in range(NH):
                    nc.vector.reciprocal_approx_fast(
                        out=recipbc[:, halves[h]], in_=lps[h][:, :])

                # attn@U (contract over j) -> projected output; normalize and
                # add the residual on the way out
                fin_all = fpool.tile([128, CT, N], F32, tag="f", name="f")
                for d in range(CT):
                    ds_ = slice(128 * d, 128 * (d + 1))
                    aps = [psA.tile([128, 512], F32, tag="psA", name="psA")
                           for _ in range(NH)]
                    for u in range(NT // 2):
                        us = slice(2 * u, 2 * u + 2)
                        for h in range(NH):
                            nc.tensor.matmul(
                                aps[h][:, :], uT[:, us, ds_],
                                expT[:, us, halves[h]],
                                start=(u == 0), stop=(u == NT // 2 - 1),
                                perf_mode=DR,
                            )
                    for h in range(NH):
                        nc.vector.tensor_mul(fin_all[:, d, halves[h]],
                                             aps[h][:, :],
                                             recipbc[:, halves[h]])
                        nc.vector.tensor_add(
                            fin_all[:, d, halves[h]], fin_all[:, d, halves[h]],
                            xt[d].bitcast(F32)[:, halves[h]],
                        )
                        nc.sync.dma_start(
                            out=out_d[img, 128 * d:128 * (d + 1), halves[h]],
                            in_=fin_all[:, d, halves[h]],
                        )

            def _body():
                seq = [i % BPC for i in range(BPC)]
                states = [None] * len(seq)
                states[0] = emit_gn(seq[0], x_pre=x0 if seq[0] == 0 else None)
                emit_mu(states[0])
                for i, img in enumerate(seq):
                    if i + 1 < len(seq):
                        states[i + 1] = emit_gn(seq[i + 1])
                    emit_att(states[i])
                    states[i] = None
                    if i + 1 < len(seq):
                        emit_mu(states[i + 1])

            if loop_iters:
                with tc.For_i(0, loop_iters, 1,
                              hint_engines=(mybir.EngineType.PE,
                                            mybir.EngineType.Activation,
                                            mybir.EngineType.DVE,
                                            mybir.EngineType.SP)):
                    _body()
            else:
                _body()

    _dedup_ldweights(nc)
    nc.compile()
    return nc


def _to_f8(a):
    return np.ascontiguousarray(
        np.clip(a, -240.0, 240.0).astype(ml_dtypes.float8_e4m3)
    )


def _prep_inputs(x, gn_scale, gn_bias, qkv_w, qkv_b, proj_w, proj_b):
    f = np.float32
    x_r = np.asarray(x, dtype=f).reshape(B, C, N)
    qkv_w = np.asarray(qkv_w, dtype=f)
    qkv_b = np.asarray(qkv_b, dtype=f)
    proj_w = np.asarray(proj_w, dtype=f)
    proj_b = np.asarray(proj_b, dtype=f)
    if np.any(qkv_b[0:2 * C]):
        raise NotImplementedError(
            "fused-weights kernel assumes zero q/k biases (reference uses zeros)"
        )
    # v-bias and proj-bias fold into a constant per-channel offset added to x
    # (rows of attn sum to 1): out += Wp @ bv + bp.
    bv = qkv_b[2 * C:3 * C]
    cvec = proj_w @ bv + proj_b
    if np.any(cvec):
        x_r = x_r + cvec[None, :, None]

    def col(v):
        return np.asarray(v, f).reshape(CT, 128).T

    consts = np.concatenate([col(gn_scale), col(gn_bias)], axis=1)
    indicator = (np.arange(C)[:, None] // GS == np.arange(G)[None, :]).astype(f)
    M = qkv_w[0:C].T @ qkv_w[C:2 * C]   # s_ij = xn_i^T M xn_j
    Wpr = proj_w @ qkv_w[2 * C:3 * C]   # u = W' xn
    common = {
        "wm": _to_f8(M * WSC),          # stationary [c,o]: m = wm^T xn = M^T xn
        "wu": _to_f8(Wpr.T * WSC),      # stationary [c,o]: u = wu^T xn = W' xn
        "ind": np.ascontiguousarray(indicator / (GS * N)),
        "bind": np.ascontiguousarray(indicator.T),
        "onesm": np.full((128, 256), 16.0, dtype=ml_dtypes.float8_e4m3),
        "consts": np.ascontiguousarray(consts),
    }
    in_maps = []
    for i in range(NCORES):
        m = dict(common)
        m["x"] = np.ascontiguousarray(x_r[BPC * i:BPC * (i + 1)])
        in_maps.append(m)
    return in_maps, True


def kernel(x, gn_scale, gn_bias, qkv_w, qkv_b, proj_w, proj_b, _trace=False):
    in_maps, _ = _prep_inputs(x, gn_scale, gn_bias, qkv_w, qkv_b,
                              proj_w, proj_b)
    if "nc" not in _cache:
        _cache["nc"] = _build()
    nc = _cache["nc"]
    res = run_bass_kernel_spmd(nc, in_maps, core_ids=list(range(NCORES)),
                               trace=_trace)
    _cache["last_result"] = res
    out = np.stack([r["out"] for r in res.results], axis=0)
    return out.reshape(B, C, H, W)



# revision 29
# speedup vs baseline: 1.2486x; 1.2486x over previous
"""AttentionBlock (GroupNorm(32) + 1-head self-attention + proj + residual) on 8 trn2 cores.

Data-parallel over batch: each of the 8 NeuronCores processes 2 of the 16 images.

Algebraic fusion (valid because the reference's q/k biases are zero):
  scores: s_ij = q_i.k_j = xn_i^T (Wq^T Wk) xn_j. Precompute M = Wq^T Wk
          host-side, compute m = M^T xn on device (ONE projection instead of
          q and k), then s_ij = m_i . xn_j with xn as the stationary operand.
  output: proj(attn@V) = sum_j p_j (Wp Wv xn_j). Precompute W' = Wp Wv, so
          u = W' xn replaces v and the separate projection matmul vanishes.

All large matmuls run in fp8e4 with perf_mode=DoubleRow (K=256 per pass).
Scaling scheme: M and W' are scaled by 16 host-side; the softmax scale c^-0.5
and the 1/16 fold into the exp activation's scale; a -2.0 bias keeps exp
outputs <= ~40 (cancels in softmax). u carries 16x, which cancels against the
all-16s matmul used for the softmax denominator (recip of 16*l).

Engine balancing (v2): GroupNorm statistics via DVE bn_stats/bn_aggr (one
pass over x, no PSUM stats matmuls, no x^2 materialization); per-channel
(mean, E[x^2]) reduced to groups by tiny f32r indicator matmuls. PSUM tiles
are [128,1024] (2 banks) so PSUM->SBUF evacuations and exp run as single
1024-wide instructions. Evacuations and the fin tail are split across
ACT/DVE/Pool (gpsimd) per the engine-assignment tables below; Pool (which
cannot touch PSUM) takes the SBUF-only work (xn quantize, residual adds) and
was idle in the baseline.

Scheduling (v3): the For_i timing loop is software-pipelined and 8x
unrolled. Per-image state (x, xn, m, uT) exists in two generations; each
unrolled step runs attention on one generation while producing the other
(x load -> bn_stats -> rstd chain -> xn -> m/u projections), so every
engine's queue opens with work whose inputs are already resident. Within a
step the two images' tile streams are interleaved instruction-by-instruction
so semaphore latency on one stream hides under the other's compute. Input
DMAs ride the SP queue; output DMAs ride the ACT hwdge queue.
"""

import ml_dtypes
import numpy as np

import concourse.bacc as bacc
import concourse.tile as tile
import concourse.mybir as mybir
from concourse.bass_utils import run_bass_kernel_spmd

F32 = mybir.dt.float32
F32R = mybir.dt.float32r
F8 = mybir.dt.float8e4
I32 = mybir.dt.int32
AF = mybir.ActivationFunctionType
ALU = mybir.AluOpType
AX = mybir.AxisListType
DR = mybir.MatmulPerfMode.DoubleRow

B, C, H, W = 16, 512, 32, 32
N = H * W                 # 1024 positions
NCORES = 8
BPC = B // NCORES         # 2 images per core
G = 32                    # groupnorm groups
GS = C // G               # 16 channels per group
CT = C // 128             # 4 channel tiles
NT = N // 128             # 8 position tiles
EPS = 1e-5
SCALE = float(C) ** -0.5  # single head, head_dim = C
WSC = 16.0                # host-side weight scale (power of 2, exact in fp8)
EXP_SCALE = SCALE / WSC   # m carries 16x; exp undoes it + softmax scale
EXP_BIAS = -2.0
MAGIC = 0x5F3759DF        # Newton-rsqrt seed constant

# engine assignment tables (tuned against TimelineSim). GPSIMD (Pool) cannot
# touch PSUM, so PSUM evacuations split across ACT/DVE and Pool gets the
# SBUF-only work (xn quantize, residual adds).
UT_COPY_ENG = ["act", "act", "act", "dve"]      # per n-pair
FINADD_ENG = ["pool", "pool", "pool", "dve"]    # per d-tile
XN_ENG = "pool"
BODY_ORDER = "v1"   # v1: ATT(a),PROD(a),ATT(b),PROD(b); v2: ATT,ATT,PROD,PROD

_cache: dict = {}


def _dedup_ldweights(nc):
    """Drop InstLdweights that reload the identical weights AP."""
    ndrop = 0
    for f in nc.m.functions:
        for blk in f.blocks:
            insts = list(blk.instructions)
            drop = []
            last_key = None
            for idx, inst in enumerate(insts):
                nm = type(inst).__name__
                if nm == "InstLdweights":
                    si = inst.sync_info
                    has_sync = si is not None and (
                        len(si.on_wait) > 0 or len(si.on_update) > 0
                    )
                    key = str(inst.ins[0])
                    if key == last_key and not has_sync:
                        drop.append(idx)
                    else:
                        last_key = key
                elif nm == "InstMatmult":
                    if inst.perf_mode is None:
                        last_key = None
            for idx in reversed(drop):
                del blk.instructions[idx]
            ndrop += len(drop)
    return ndrop


def _build(loop_iters: int = 0):
    nc = bacc.Bacc("TRN2", target_bir_lowering=False, num_devices=NCORES)

    x_d = nc.dram_tensor("x", [BPC, C, N], F32R, kind="ExternalInput")
    wm_d = nc.dram_tensor("wm", [C, C], F8, kind="ExternalInput")   # 16*(Wq^T Wk)
    wu_d = nc.dram_tensor("wu", [C, C], F8, kind="ExternalInput")   # 16*(Wp Wv)^T
    ind_d = nc.dram_tensor("ind16", [C, G], F32R, kind="ExternalInput")  # 1/16 iff c//16==g
    bind_d = nc.dram_tensor("bind", [G, C], F32R, kind="ExternalInput")  # 0/1 indicator.T
    ones_d = nc.dram_tensor("onesm", [128, 256], F8, kind="ExternalInput")  # all 16.0
    consts_d = nc.dram_tensor("consts", [128, 2 * CT], F32, kind="ExternalInput")
    out_d = nc.dram_tensor("out", [BPC, C, N], F32, kind="ExternalOutput")

    halves = [slice(0, 512), slice(512, 1024)]

    with tile.TileContext(nc) as tc:
        with (
            tc.tile_pool(name="wpool", bufs=1) as wp_,
            tc.tile_pool(name="state", bufs=1) as stp,
            tc.tile_pool(name="epool", bufs=2) as epool,
            tc.tile_pool(name="fpool", bufs=2) as fpool,
            tc.tile_pool(name="rpool", bufs=2) as rpool,
            tc.tile_pool(name="spool", bufs=2) as spool,
            tc.tile_pool(name="ps2", bufs=2, space="PSUM") as ps2,
            tc.tile_pool(name="psA", bufs=3, space="PSUM") as psA,
            tc.tile_pool(name="psT", bufs=1, space="PSUM") as psT,
        ):
            # ---- persistent constants / weights (batched single DMAs) ----
            wm_all = wp_.tile([128, CT, C], F8, tag="wm", name="wm")
            wu_all = wp_.tile([128, CT, C], F8, tag="wu", name="wu")
            ind_all = wp_.tile([128, CT, G], F32R, tag="ind", name="ind")
            bind_all = wp_.tile([G, CT, 128], F32R, tag="bind", name="bind")
            ones_sb = wp_.tile([128, 2, 128], F8, tag="ones", name="ones")
            consts_sb = wp_.tile([128, 2 * CT], F32, tag="consts", name="consts")
            magic_sb = wp_.tile([128, 1], I32, tag="magic", name="magic")
            nc.vector.memset(magic_sb, MAGIC)
            exp_sc = wp_.tile([128, 1], F32, tag="expsc", name="expsc")
            nc.vector.memset(exp_sc, EXP_SCALE)
            exp_bi = wp_.tile([128, 1], F32, tag="expbi", name="expbi")
            nc.vector.memset(exp_bi, EXP_BIAS)
            gnsc_sb = consts_sb[:, 0 * CT:1 * CT]
            gnbi_sb = consts_sb[:, 1 * CT:2 * CT]

            def part(dram2d):
                return dram2d.rearrange("(t p) f -> p t f", p=128)

            nc.sync.dma_start(out=ind_all, in_=part(ind_d[:, :]))
            nc.sync.dma_start(out=consts_sb, in_=consts_d[:, :])
            nc.sync.dma_start(
                out=bind_all, in_=bind_d.rearrange("g (t p) -> g t p", p=128)
            )
            nc.sync.dma_start(out=wm_all, in_=part(wm_d[:, :]))
            nc.sync.dma_start(out=wu_all, in_=part(wu_d[:, :]))
            nc.sync.dma_start(
                out=ones_sb, in_=ones_d.rearrange("p (s f) -> p s f", s=2)
            )

            # per-(image, generation) persistent state: PROD writes one
            # generation while ATT consumes the other; the For_i body is
            # unrolled 2x so the generations alternate with static buffers.
            def mkstate(s):
                return {
                    "x": stp.tile([128, CT, N], F32R, tag=f"x{s}", name=f"x{s}"),
                    "xn": stp.tile([128, CT, N], F8, tag=f"xn{s}", name=f"xn{s}"),
                    "m": stp.tile([128, CT, N], F8, tag=f"m{s}", name=f"m{s}"),
                    "uT": stp.tile([128, NT, C], F8, tag=f"u{s}", name=f"u{s}"),
                }
            gens = [[mkstate("a0"), mkstate("b0")],
                    [mkstate("a1"), mkstate("b1")]]
            for g in gens:
                for i, st in enumerate(g):
                    st["img"] = i
            states = gens[0]

            def eng(name):
                return {"act": nc.scalar, "dve": nc.vector, "pool": nc.gpsimd}[name]

            def emit_gn(st):
                """bn_stats -> group agg -> a/b coefficients for one image.
                Small chain; caller interleaves the two images."""
                x_all = st["x"]
                xf = x_all.bitcast(F32)
                bst = spool.tile([128, 2 * CT, 6], F32, tag="bst", name="bst")
                for t in range(CT):
                    for h in range(2):
                        nc.vector.bn_stats(
                            out=bst[:, 2 * t + h, :], in_=xf[:, t, halves[h]]
                        )
                mv = spool.tile([128, CT, 2], F32, tag="mv", name="mv")
                for t in range(CT):
                    nc.vector.bn_aggr(
                        out=mv[:, t, :], in_=bst[:, 2 * t:2 * t + 2, :]
                    )
                msq = spool.tile([128, CT], F32, tag="msq", name="msq")
                nc.vector.tensor_mul(msq[:, :], mv[:, :, 0], mv[:, :, 0])
                m12 = spool.tile([128, CT, 2], F32R, tag="m12", name="m12")
                nc.vector.tensor_copy(m12[:, :, 0], mv[:, :, 0])
                nc.vector.tensor_add(m12[:, :, 1], mv[:, :, 1], msq[:, :])
                tiny = psT.tile([128, 16], F32, tag="tiny", name="tiny")
                gps = tiny[0:G, 0:2]
                for t in range(CT):
                    nc.tensor.matmul(
                        gps[:, :], ind_all[:, t, :], m12[:, t, :],
                        start=(t == 0), stop=(t == CT - 1),
                    )
                meang = spool.tile([G, 1], F32, tag="meang", name="meang")
                nc.vector.tensor_copy(meang[:, :], gps[:, 0:1])
                msqg = spool.tile([G, 1], F32, tag="msqg", name="msqg")
                vpe = spool.tile([G, 1], F32, tag="vpe", name="vpe")
                nc.vector.tensor_mul(msqg[:, :], meang[:, :], meang[:, :])
                nc.vector.scalar_tensor_tensor(
                    out=vpe[:, :], in0=gps[:, 1:2], scalar=EPS, in1=msqg[:, :],
                    op0=ALU.add, op1=ALU.subtract,
                )
                sh_t = spool.tile([G, 1], I32, tag="sh", name="sh")
                nc.vector.tensor_scalar(
                    out=sh_t[:, :], in0=vpe.bitcast(I32)[:, :], scalar1=1,
                    scalar2=None, op0=ALU.logical_shift_right,
                )
                seed = spool.tile([G, 1], I32, tag="seed", name="seed")
                nc.vector.scalar_tensor_tensor(
                    out=seed[:, :], in0=magic_sb[:G, :], scalar=0, in1=sh_t[:, :],
                    op0=ALU.bypass, op1=ALU.subtract,
                )
                y = seed.bitcast(F32)
                for it in range(2):
                    t1 = spool.tile([G, 1], F32, tag=f"nr{it}", name=f"nr{it}")
                    nc.vector.tensor_mul(t1[:, :], y[:, :], y[:, :])
                    nc.vector.tensor_mul(t1[:, :], t1[:, :], vpe[:, :])
                    nc.vector.tensor_scalar(
                        out=t1[:, :], in0=t1[:, :], scalar1=-0.5, scalar2=1.5,
                        op0=ALU.mult, op1=ALU.add,
                    )
                    y2 = spool.tile([G, 1], F32, tag=f"y{it}", name=f"y{it}")
                    nc.vector.tensor_mul(y2[:, :], y[:, :], t1[:, :])
                    y = y2
                stats2 = spool.tile([G, 2], F32R, tag="st2", name="st2")
                nc.vector.tensor_copy(stats2[:, 0:1], y[:, :])
                nc.vector.tensor_copy(stats2[:, 1:2], meang[:, :])
                bc_ps = tiny[:, 8:8 + 2 * CT]
                for t in range(CT):
                    nc.tensor.matmul(
                        bc_ps[:, 2 * t:2 * t + 2], bind_all[:, t, :], stats2[:, :],
                        start=True, stop=True,
                    )
                bc_v = bc_ps.rearrange("p (t s) -> p t s", s=2)
                a_all = spool.tile([128, CT], F32, tag="aall", name="aall")
                b_all = spool.tile([128, CT], F32, tag="ball", name="ball")
                nc.vector.tensor_mul(a_all[:, :], gnsc_sb, bc_v[:, :, 0])
                nc.vector.scalar_tensor_tensor(
                    out=b_all[:, :], in0=bc_v[:, :, 1], scalar=-1.0, in1=a_all[:, :],
                    op0=ALU.mult, op1=ALU.mult,
                )
                nc.vector.tensor_add(b_all[:, :], b_all[:, :], gnbi_sb)
                return a_all, b_all

            def prod_load(st0, st1):
                """x loads for both images, emitted at body start so the SP
                queue issues them before the fin-gated out DMAs."""
                for st in (st0, st1):
                    x_all = st["x"]
                    for t in range(CT):
                        for h in range(2):
                            nc.sync.dma_start(
                                out=x_all[:, t, halves[h]],
                                in_=x_d[st["img"], 128 * t:128 * (t + 1),
                                        halves[h]],
                            )

            def prod_gn(st0, st1):
                sts = [st0, st1]
                abs_ = [emit_gn(st0), emit_gn(st1)]
                for t in range(CT):
                    for s, st in enumerate(sts):
                        a_all, b_all = abs_[s]
                        eng(XN_ENG).tensor_scalar(
                            out=st["xn"][:, t, :],
                            in0=st["x"].bitcast(F32)[:, t, :],
                            scalar1=a_all[:, t:t + 1],
                            scalar2=b_all[:, t:t + 1],
                            op0=ALU.mult, op1=ALU.add,
                        )

            def prod_mu(st0, st1):
                sts = [st0, st1]
                for d in range(CT):
                    ds_ = slice(128 * d, 128 * (d + 1))
                    for st in sts:
                        ps = ps2.tile([128, N], F32, tag="ps2", name="ps2")
                        for u in range(CT // 2):
                            us = slice(2 * u, 2 * u + 2)
                            for h in range(2):
                                nc.tensor.matmul(
                                    ps[:, halves[h]], wm_all[:, us, ds_],
                                    st["xn"][:, us, halves[h]],
                                    start=(u == 0), stop=(u == CT // 2 - 1),
                                    perf_mode=DR,
                                )
                        nc.scalar.copy(out=st["m"][:, d, :], in_=ps[:, :])
                for np_ in range(NT // 2):
                    for st in sts:
                        ps = ps2.tile([128, N], F32, tag="ps2", name="ps2")
                        psv = ps.rearrange("p (k f) -> p k f", k=2)
                        for k in range(2):
                            n = 2 * np_ + k
                            ns = slice(128 * n, 128 * (n + 1))
                            for u in range(CT // 2):
                                us = slice(2 * u, 2 * u + 2)
                                nc.tensor.matmul(
                                    psv[:, k, :], st["xn"][:, us, ns],
                                    wu_all[:, us, :],
                                    start=(u == 0), stop=(u == CT // 2 - 1),
                                    perf_mode=DR,
                                )
                        e = UT_COPY_ENG[np_]
                        if e == "act":
                            nc.scalar.copy(
                                out=st["uT"][:, 2 * np_:2 * np_ + 2, :],
                                in_=psv[:, :, :],
                            )
                        else:
                            eng(e).tensor_copy(
                                st["uT"][:, 2 * np_:2 * np_ + 2, :], psv[:, :, :]
                            )

            att_ctx = {}

            def att_fwd(st0, st1):
                """S/exp/L/recip for both images, tile-interleaved."""
                sts = [st0, st1]
                exps, recips = [], []
                for st in sts:
                    exps.append(epool.tile([128, NT, N], F8, tag="e", name="e"))
                    recips.append(rpool.tile([128, N], F32, tag="rbc", name="rbc"))
                for j in range(NT):
                    js = slice(128 * j, 128 * (j + 1))
                    for s, st in enumerate(sts):
                        ps = ps2.tile([128, N], F32, tag="ps2", name="ps2")
                        for u in range(CT // 2):
                            us = slice(2 * u, 2 * u + 2)
                            for h in range(2):
                                nc.tensor.matmul(
                                    ps[:, halves[h]], st["xn"][:, us, js],
                                    st["m"][:, us, halves[h]],
                                    start=(u == 0), stop=(u == CT // 2 - 1),
                                    perf_mode=DR,
                                )
                        nc.scalar.activation(out=exps[s][:, j, :], in_=ps[:, :],
                                             func=AF.Exp, scale=exp_sc[:, 0:1],
                                             bias=exp_bi[:, 0:1])
                lps = []
                for s in range(2):
                    ps = ps2.tile([128, N], F32, tag="ps2", name="ps2")
                    for u in range(NT // 2):
                        us = slice(2 * u, 2 * u + 2)
                        for h in range(2):
                            nc.tensor.matmul(
                                ps[:, halves[h]], ones_sb[:, :, :],
                                exps[s][:, us, halves[h]],
                                start=(u == 0), stop=(u == NT // 2 - 1),
                                perf_mode=DR,
                            )
                    lps.append(ps)
                for s in range(2):
                    nc.vector.reciprocal_approx_fast(out=recips[s][:, :],
                                                     in_=lps[s][:, :])
                att_ctx["exps"], att_ctx["recips"] = exps, recips

            def att_bwd(st0, st1):
                """attn@U, normalize, residual, writeback for both images."""
                sts = [st0, st1]
                exps, recips = att_ctx["exps"], att_ctx["recips"]
                for d in range(CT):
                    ds_ = slice(128 * d, 128 * (d + 1))
                    fins = []
                    for s, st in enumerate(sts):
                        fin = fpool.tile([128, N], F32, tag="fin", name="fin")
                        fins.append(fin)
                        for h in range(2):
                            ps = psA.tile([128, 512], F32, tag="psA", name="psA")
                            for u in range(NT // 2):
                                us = slice(2 * u, 2 * u + 2)
                                nc.tensor.matmul(
                                    ps[:, :], st["uT"][:, us, ds_],
                                    exps[s][:, us, halves[h]],
                                    start=(u == 0), stop=(u == NT // 2 - 1),
                                    perf_mode=DR,
                                )
                            nc.vector.tensor_mul(fins[s][:, halves[h]], ps[:, :],
                                                 recips[s][:, halves[h]])
                    for s, st in enumerate(sts):
                        xf = st["x"].bitcast(F32)
                        eng(FINADD_ENG[d]).tensor_add(
                            fins[s][:, :], fins[s][:, :], xf[:, d, :])
                        nc.scalar.dma_start(
                            out=out_d[st["img"], 128 * d:128 * (d + 1), :],
                            in_=fins[s][:, :],
                        )

            if loop_iters:
                # software-pipelined + 2x unrolled: the prologue produces
                # generation 0; each unrolled half consumes one generation
                # while producing the other, so no tight write-after-read
                # cycles on the state buffers. Total invocations = loop_iters.
                UNROLL = 8   # invocations per For_i iteration (amortizes
                             # the all-engine barrier at the loop back edge)
                assert loop_iters % UNROLL == 0
                prod_load(*gens[0])
                prod_gn(*gens[0])
                prod_mu(*gens[0])
                with tc.For_i(0, loop_iters // UNROLL, 1,
                              hint_engines=(mybir.EngineType.PE,
                                            mybir.EngineType.Activation,
                                            mybir.EngineType.DVE,
                                            mybir.EngineType.Pool,
                                            mybir.EngineType.SP)):
                    for rep in range(UNROLL // 2):
                        for g in (0, 1):
                            att_fwd(*gens[g])
                            att_bwd(*gens[g])
                            prod_load(*gens[1 - g])
                            prod_gn(*gens[1 - g])
                            prod_mu(*gens[1 - g])
            else:
                prod_load(*gens[0])
                prod_gn(*gens[0])
                prod_mu(*gens[0])
                att_fwd(*gens[0])
                att_bwd(*gens[0])

    _dedup_ldweights(nc)
    nc.compile()
    return nc


def _to_f8(a):
    return np.ascontiguousarray(
        np.clip(a, -240.0, 240.0).astype(ml_dtypes.float8_e4m3)
    )


def _prep_inputs(x, gn_scale, gn_bias, qkv_w, qkv_b, proj_w, proj_b):
    f = np.float32
    x_r = np.asarray(x, dtype=f).reshape(B, C, N)
    qkv_w = np.asarray(qkv_w, dtype=f)
    qkv_b = np.asarray(qkv_b, dtype=f)
    proj_w = np.asarray(proj_w, dtype=f)
    proj_b = np.asarray(proj_b, dtype=f)
    if np.any(qkv_b[0:2 * C]):
        raise NotImplementedError(
            "fused-weights kernel assumes zero q/k biases (reference uses zeros)"
        )
    # v-bias and proj-bias fold into a constant per-channel offset added to x
    # (rows of attn sum to 1): out += Wp @ bv + bp.
    bv = qkv_b[2 * C:3 * C]
    cvec = proj_w @ bv + proj_b
    if np.any(cvec):
        x_r = x_r + cvec[None, :, None]

    def col(v):
        return np.asarray(v, f).reshape(CT, 128).T

    consts = np.concatenate([col(gn_scale), col(gn_bias)], axis=1)
    indicator = (np.arange(C)[:, None] // GS == np.arange(G)[None, :]).astype(f)
    M = qkv_w[0:C].T @ qkv_w[C:2 * C]   # s_ij = xn_i^T M xn_j
    Wpr = proj_w @ qkv_w[2 * C:3 * C]   # u = W' xn
    common = {
        "wm": _to_f8(M * WSC),          # stationary [c,o]: m = wm^T xn = M^T xn
        "wu": _to_f8(Wpr.T * WSC),      # stationary [c,o]: u = wu^T xn = W' xn
        "ind16": np.ascontiguousarray(indicator / GS),
        "bind": np.ascontiguousarray(indicator.T),
        "onesm": np.full((128, 256), 16.0, dtype=ml_dtypes.float8_e4m3),
        "consts": np.ascontiguousarray(consts),
    }
    in_maps = []
    for i in range(NCORES):
        m = dict(common)
        m["x"] = np.ascontiguousarray(x_r[BPC * i:BPC * (i + 1)])
        in_maps.append(m)
    return in_maps, True


def kernel(x, gn_scale, gn_bias, qkv_w, qkv_b, proj_w, proj_b, _trace=False):
    in_maps, _ = _prep_inputs(x, gn_scale, gn_bias, qkv_w, qkv_b,
                              proj_w, proj_b)
    if "nc" not in _cache:
        _cache["nc"] = _build()
    nc = _cache["nc"]
    res = run_bass_kernel_spmd(nc, in_maps, core_ids=list(range(NCORES)),
                               trace=_trace)
    _cache["last_result"] = res
    out = np.stack([r["out"] for r in res.results], axis=0)
    return out.reshape(B, C, H, W)


# revision 34
# speedup vs baseline: 1.3100x; 1.0491x over previous
"""AttentionBlock (GroupNorm(32) + 1-head self-attention + proj + residual) on 8 trn2 cores.

Data-parallel over batch: each of the 8 NeuronCores processes 2 of the 16 images.

Algebraic fusion (valid because the reference's q/k biases are zero):
  scores: s_ij = q_i.k_j = xn_i^T (Wq^T Wk) xn_j. Precompute M = Wq^T Wk
          host-side, compute m = M^T xn on device (ONE projection instead of
          q and k), then s_ij = m_i . xn_j with xn as the stationary operand.
  output: proj(attn@V) = sum_j p_j (Wp Wv xn_j). Precompute W' = Wp Wv, so
          u = W' xn replaces v and the separate projection matmul vanishes.

All large matmuls run in fp8e4 with perf_mode=DoubleRow (K=256 per pass).
Scaling scheme: M and W' are scaled by 16 host-side; the softmax scale c^-0.5
and the 1/16 fold into the exp activation's scale; a -2.0 bias keeps exp
outputs <= ~40 (cancels in softmax). u carries 16x, which cancels against the
all-16s matmul used for the softmax denominator (recip of 16*l).

Engine balancing (v2): GroupNorm statistics via DVE bn_stats/bn_aggr (one
pass over x, no PSUM stats matmuls, no x^2 materialization); per-channel
(mean, E[x^2]) reduced to groups by tiny f32r indicator matmuls. PSUM tiles
are [128,1024] (2 banks) so PSUM->SBUF evacuations and exp run as single
1024-wide instructions. Evacuations and the fin tail are split across
ACT/DVE/Pool (gpsimd) per the engine-assignment tables below; Pool was idle
in the baseline. Emission interleaves the two images so ACT (exp+xn, the
busiest engine) stays fed: img b's bn_stats run under img a's attention, and
img a's attn@U runs while ACT converts xn_b.
"""

import ml_dtypes
import numpy as np

import concourse.bacc as bacc
import concourse.tile as tile
import concourse.mybir as mybir
from concourse.bass_utils import run_bass_kernel_spmd

F32 = mybir.dt.float32
F32R = mybir.dt.float32r
F8 = mybir.dt.float8e4
I32 = mybir.dt.int32
AF = mybir.ActivationFunctionType
ALU = mybir.AluOpType
AX = mybir.AxisListType
DR = mybir.MatmulPerfMode.DoubleRow

B, C, H, W = 16, 512, 32, 32
N = H * W                 # 1024 positions
NCORES = 8
BPC = B // NCORES         # 2 images per core
G = 32                    # groupnorm groups
GS = C // G               # 16 channels per group
CT = C // 128             # 4 channel tiles
NT = N // 128             # 8 position tiles
EPS = 1e-5
SCALE = float(C) ** -0.5  # single head, head_dim = C
WSC = 16.0                # host-side weight scale (power of 2, exact in fp8)
EXP_SCALE = SCALE / WSC   # m carries 16x; exp undoes it + softmax scale
EXP_BIAS = -2.0
MAGIC = 0x5F3759DF        # Newton-rsqrt seed constant

# engine assignment tables (tuned against TimelineSim). GPSIMD (Pool) cannot
# touch PSUM, so PSUM evacuations split across ACT/DVE and Pool gets the
# SBUF-only work (xn quantize, residual adds).
UT_COPY_ENG = ["act", "act", "act", "dve"]      # per n-pair
FINADD_ENG = ["pool", "pool", "pool", "dve"]    # per d-tile
XN_ENG = "pool"
BODY_ORDER = "v1"   # v1: ATT(a),PROD(a),ATT(b),PROD(b); v2: ATT,ATT,PROD,PROD

_cache: dict = {}


def _dedup_ldweights(nc):
    """Drop InstLdweights that reload the identical weights AP."""
    ndrop = 0
    for f in nc.m.functions:
        for blk in f.blocks:
            insts = list(blk.instructions)
            drop = []
            last_key = None
            for idx, inst in enumerate(insts):
                nm = type(inst).__name__
                if nm == "InstLdweights":
                    si = inst.sync_info
                    has_sync = si is not None and (
                        len(si.on_wait) > 0 or len(si.on_update) > 0
                    )
                    key = str(inst.ins[0])
                    if key == last_key and not has_sync:
                        drop.append(idx)
                    else:
                        last_key = key
                elif nm == "InstMatmult":
                    if inst.perf_mode is None:
                        last_key = None
            for idx in reversed(drop):
                del blk.instructions[idx]
            ndrop += len(drop)
    return ndrop


def _build(loop_iters: int = 0):
    nc = bacc.Bacc("TRN2", target_bir_lowering=False, num_devices=NCORES)

    x_d = nc.dram_tensor("x", [BPC, C, N], F32R, kind="ExternalInput")
    wm_d = nc.dram_tensor("wm", [C, C], F8, kind="ExternalInput")   # 16*(Wq^T Wk)
    wu_d = nc.dram_tensor("wu", [C, C], F8, kind="ExternalInput")   # 16*(Wp Wv)^T
    ind_d = nc.dram_tensor("ind16", [C, G], F32R, kind="ExternalInput")  # 1/16 iff c//16==g
    bind_d = nc.dram_tensor("bind", [G, C], F32R, kind="ExternalInput")  # 0/1 indicator.T
    ones_d = nc.dram_tensor("onesm", [128, 256], F8, kind="ExternalInput")  # all 16.0
    consts_d = nc.dram_tensor("consts", [128, 2 * CT], F32, kind="ExternalInput")
    out_d = nc.dram_tensor("out", [BPC, C, N], F32, kind="ExternalOutput")

    halves = [slice(0, 512), slice(512, 1024)]

    with tile.TileContext(nc) as tc:
        with (
            tc.tile_pool(name="wpool", bufs=1) as wp_,
            tc.tile_pool(name="state", bufs=1) as stp,
            tc.tile_pool(name="epool", bufs=2) as epool,
            tc.tile_pool(name="fpool", bufs=2) as fpool,
            tc.tile_pool(name="rpool", bufs=2) as rpool,
            tc.tile_pool(name="spool", bufs=2) as spool,
            tc.tile_pool(name="ps2", bufs=2, space="PSUM") as ps2,
            tc.tile_pool(name="psA", bufs=3, space="PSUM") as psA,
            tc.tile_pool(name="psT", bufs=1, space="PSUM") as psT,
        ):
            # ---- persistent constants / weights (batched single DMAs) ----
            wm_all = wp_.tile([128, CT, C], F8, tag="wm", name="wm")
            wu_all = wp_.tile([128, CT, C], F8, tag="wu", name="wu")
            ind_all = wp_.tile([128, CT, G], F32R, tag="ind", name="ind")
            bind_all = wp_.tile([G, CT, 128], F32R, tag="bind", name="bind")
            ones_sb = wp_.tile([128, 2, 128], F8, tag="ones", name="ones")
            consts_sb = wp_.tile([128, 2 * CT], F32, tag="consts", name="consts")
            magic_sb = wp_.tile([128, 1], I32, tag="magic", name="magic")
            nc.vector.memset(magic_sb, MAGIC)
            exp_sc = wp_.tile([128, 1], F32, tag="expsc", name="expsc")
            nc.vector.memset(exp_sc, EXP_SCALE)
            exp_bi = wp_.tile([128, 1], F32, tag="expbi", name="expbi")
            nc.vector.memset(exp_bi, EXP_BIAS)
            gnsc_sb = consts_sb[:, 0 * CT:1 * CT]
            gnbi_sb = consts_sb[:, 1 * CT:2 * CT]

            def part(dram2d):
                return dram2d.rearrange("(t p) f -> p t f", p=128)

            nc.sync.dma_start(out=ind_all, in_=part(ind_d[:, :]))
            nc.sync.dma_start(out=consts_sb, in_=consts_d[:, :])
            nc.sync.dma_start(
                out=bind_all, in_=bind_d.rearrange("g (t p) -> g t p", p=128)
            )
            nc.sync.dma_start(out=wm_all, in_=part(wm_d[:, :]))
            nc.sync.dma_start(out=wu_all, in_=part(wu_d[:, :]))
            nc.sync.dma_start(
                out=ones_sb, in_=ones_d.rearrange("p (s f) -> p s f", s=2)
            )

            # per-(image, generation) persistent state: PROD writes one
            # generation while ATT consumes the other; the For_i body is
            # unrolled 2x so the generations alternate with static buffers.
            def mkstate(s):
                return {
                    "x": stp.tile([128, CT, N], F32R, tag=f"x{s}", name=f"x{s}"),
                    "xn": stp.tile([128, CT, N], F8, tag=f"xn{s}", name=f"xn{s}"),
                    "m": stp.tile([128, CT, N], F8, tag=f"m{s}", name=f"m{s}"),
                    "uT": stp.tile([128, NT, C], F8, tag=f"u{s}", name=f"u{s}"),
                }
            gens = [[mkstate("a0"), mkstate("b0")],
                    [mkstate("a1"), mkstate("b1")]]
            for g in gens:
                for i, st in enumerate(g):
                    st["img"] = i
            states = gens[0]

            def eng(name):
                return {"act": nc.scalar, "dve": nc.vector, "pool": nc.gpsimd}[name]

            def emit_gn(st):
                """bn_stats -> group agg -> a/b coefficients for one image.
                Small chain; caller interleaves the two images."""
                x_all = st["x"]
                xf = x_all.bitcast(F32)
                bst = spool.tile([128, 2 * CT, 6], F32, tag="bst", name="bst")
                for t in range(CT):
                    for h in range(2):
                        nc.vector.bn_stats(
                            out=bst[:, 2 * t + h, :], in_=xf[:, t, halves[h]]
                        )
                mv = spool.tile([128, CT, 2], F32, tag="mv", name="mv")
                for t in range(CT):
                    nc.vector.bn_aggr(
                        out=mv[:, t, :], in_=bst[:, 2 * t:2 * t + 2, :]
                    )
                msq = spool.tile([128, CT], F32, tag="msq", name="msq")
                nc.vector.tensor_mul(msq[:, :], mv[:, :, 0], mv[:, :, 0])
                m12 = spool.tile([128, CT, 2], F32R, tag="m12", name="m12")
                nc.vector.tensor_copy(m12[:, :, 0], mv[:, :, 0])
                nc.vector.tensor_add(m12[:, :, 1], mv[:, :, 1], msq[:, :])
                tiny = psT.tile([128, 16], F32, tag="tiny", name="tiny")
                gps = tiny[0:G, 0:2]
                for t in range(CT):
                    nc.tensor.matmul(
                        gps[:, :], ind_all[:, t, :], m12[:, t, :],
                        start=(t == 0), stop=(t == CT - 1),
                    )
                meang = spool.tile([G, 1], F32, tag="meang", name="meang")
                nc.vector.tensor_copy(meang[:, :], gps[:, 0:1])
                msqg = spool.tile([G, 1], F32, tag="msqg", name="msqg")
                vpe = spool.tile([G, 1], F32, tag="vpe", name="vpe")
                nc.vector.tensor_mul(msqg[:, :], meang[:, :], meang[:, :])
                nc.vector.scalar_tensor_tensor(
                    out=vpe[:, :], in0=gps[:, 1:2], scalar=EPS, in1=msqg[:, :],
                    op0=ALU.add, op1=ALU.subtract,
                )
                sh_t = spool.tile([G, 1], I32, tag="sh", name="sh")
                nc.vector.tensor_scalar(
                    out=sh_t[:, :], in0=vpe.bitcast(I32)[:, :], scalar1=1,
                    scalar2=None, op0=ALU.logical_shift_right,
                )
                seed = spool.tile([G, 1], I32, tag="seed", name="seed")
                nc.vector.scalar_tensor_tensor(
                    out=seed[:, :], in0=magic_sb[:G, :], scalar=0, in1=sh_t[:, :],
                    op0=ALU.bypass, op1=ALU.subtract,
                )
                y = seed.bitcast(F32)
                for it in range(2):
                    t1 = spool.tile([G, 1], F32, tag=f"nr{it}", name=f"nr{it}")
                    nc.vector.tensor_mul(t1[:, :], y[:, :], y[:, :])
                    nc.vector.tensor_mul(t1[:, :], t1[:, :], vpe[:, :])
                    nc.vector.tensor_scalar(
                        out=t1[:, :], in0=t1[:, :], scalar1=-0.5, scalar2=1.5,
                        op0=ALU.mult, op1=ALU.add,
                    )
                    y2 = spool.tile([G, 1], F32, tag=f"y{it}", name=f"y{it}")
                    nc.vector.tensor_mul(y2[:, :], y[:, :], t1[:, :])
                    y = y2
                stats2 = spool.tile([G, 2], F32R, tag="st2", name="st2")
                nc.vector.tensor_copy(stats2[:, 0:1], y[:, :])
                nc.vector.tensor_copy(stats2[:, 1:2], meang[:, :])
                bc_ps = tiny[:, 8:8 + 2 * CT]
                for t in range(CT):
                    nc.tensor.matmul(
                        bc_ps[:, 2 * t:2 * t + 2], bind_all[:, t, :], stats2[:, :],
                        start=True, stop=True,
                    )
                bc_v = bc_ps.rearrange("p (t s) -> p t s", s=2)
                a_all = spool.tile([128, CT], F32, tag="aall", name="aall")
                b_all = spool.tile([128, CT], F32, tag="ball", name="ball")
                nc.vector.tensor_mul(a_all[:, :], gnsc_sb, bc_v[:, :, 0])
                nc.vector.scalar_tensor_tensor(
                    out=b_all[:, :], in0=bc_v[:, :, 1], scalar=-1.0, in1=a_all[:, :],
                    op0=ALU.mult, op1=ALU.mult,
                )
                nc.vector.tensor_add(b_all[:, :], b_all[:, :], gnbi_sb)
                return a_all, b_all

            def prod_load(st0, st1):
                """x loads for both images, emitted at body start so the SP
                queue issues them before the fin-gated out DMAs."""
                for st in (st0, st1):
                    x_all = st["x"]
                    for t in range(CT):
                        for h in range(2):
                            nc.sync.dma_start(
                                out=x_all[:, t, halves[h]],
                                in_=x_d[st["img"], 128 * t:128 * (t + 1),
                                        halves[h]],
                            )

            def prod_gn(st0, st1):
                sts = [st0, st1]
                abs_ = [emit_gn(st0), emit_gn(st1)]
                # tiles 0-1 on DVE (immediately after the stats chain in
                # DVE's queue, unblocking the m/u projections' first pass);
                # tiles 2-3 on Pool in parallel
                for t in range(CT):
                    for s, st in enumerate(sts):
                        a_all, b_all = abs_[s]
                        e = "dve" if t < 2 else XN_ENG
                        eng(e).tensor_scalar(
                            out=st["xn"][:, t, :],
                            in0=st["x"].bitcast(F32)[:, t, :],
                            scalar1=a_all[:, t:t + 1],
                            scalar2=b_all[:, t:t + 1],
                            op0=ALU.mult, op1=ALU.add,
                        )

            def prod_mu(st0, st1):
                sts = [st0, st1]
                for d in range(CT):
                    ds_ = slice(128 * d, 128 * (d + 1))
                    for st in sts:
                        ps = ps2.tile([128, N], F32, tag="ps2", name="ps2")
                        for u in range(CT // 2):
                            us = slice(2 * u, 2 * u + 2)
                            for h in range(2):
                                nc.tensor.matmul(
                                    ps[:, halves[h]], wm_all[:, us, ds_],
                                    st["xn"][:, us, halves[h]],
                                    start=(u == 0), stop=(u == CT // 2 - 1),
                                    perf_mode=DR,
                                )
                        nc.scalar.copy(out=st["m"][:, d, :], in_=ps[:, :])
                for np_ in range(NT // 2):
                    for st in sts:
                        ps = ps2.tile([128, N], F32, tag="ps2", name="ps2")
                        psv = ps.rearrange("p (k f) -> p k f", k=2)
                        for k in range(2):
                            n = 2 * np_ + k
                            ns = slice(128 * n, 128 * (n + 1))
                            for u in range(CT // 2):
                                us = slice(2 * u, 2 * u + 2)
                                nc.tensor.matmul(
                                    psv[:, k, :], st["xn"][:, us, ns],
                                    wu_all[:, us, :],
                                    start=(u == 0), stop=(u == CT // 2 - 1),
                                    perf_mode=DR,
                                )
                        e = UT_COPY_ENG[np_]
                        if e == "act":
                            nc.scalar.copy(
                                out=st["uT"][:, 2 * np_:2 * np_ + 2, :],
                                in_=psv[:, :, :],
                            )
                        else:
                            eng(e).tensor_copy(
                                st["uT"][:, 2 * np_:2 * np_ + 2, :], psv[:, :, :]
                            )

            att_ctx = {}

            def att_fwd(st0, st1):
                """S/exp/L/recip for both images, tile-interleaved."""
                sts = [st0, st1]
                exps, recips = [], []
                for st in sts:
                    exps.append(epool.tile([128, NT, N], F8, tag="e", name="e"))
                    recips.append(rpool.tile([128, N], F32, tag="rbc", name="rbc"))
                for j in range(NT):
                    js = slice(128 * j, 128 * (j + 1))
                    for s, st in enumerate(sts):
                        ps = ps2.tile([128, N], F32, tag="ps2", name="ps2")
                        for u in range(CT // 2):
                            us = slice(2 * u, 2 * u + 2)
                            for h in range(2):
                                nc.tensor.matmul(
                                    ps[:, halves[h]], st["xn"][:, us, js],
                                    st["m"][:, us, halves[h]],
                                    start=(u == 0), stop=(u == CT // 2 - 1),
                                    perf_mode=DR,
                                )
                        nc.scalar.activation(out=exps[s][:, j, :], in_=ps[:, :],
                                             func=AF.Exp, scale=exp_sc[:, 0:1],
                                             bias=exp_bi[:, 0:1])
                lps = []
                for s in range(2):
                    ps = ps2.tile([128, N], F32, tag="ps2", name="ps2")
                    for u in range(NT // 2):
                        us = slice(2 * u, 2 * u + 2)
                        for h in range(2):
                            nc.tensor.matmul(
                                ps[:, halves[h]], ones_sb[:, :, :],
                                exps[s][:, us, halves[h]],
                                start=(u == 0), stop=(u == NT // 2 - 1),
                                perf_mode=DR,
                            )
                    lps.append(ps)
                for s in range(2):
                    nc.vector.reciprocal_approx_fast(out=recips[s][:, :],
                                                     in_=lps[s][:, :])
                att_ctx["exps"], att_ctx["recips"] = exps, recips

            def att_bwd(st0, st1):
                """attn@U, normalize, residual, writeback for both images."""
                sts = [st0, st1]
                exps, recips = att_ctx["exps"], att_ctx["recips"]
                for d in range(CT):
                    ds_ = slice(128 * d, 128 * (d + 1))
                    fins = []
                    for s, st in enumerate(sts):
                        fin = fpool.tile([128, N], F32, tag="fin", name="fin")
                        fins.append(fin)
                        for h in range(2):
                            ps = psA.tile([128, 512], F32, tag="psA", name="psA")
                            for u in range(NT // 2):
                                us = slice(2 * u, 2 * u + 2)
                                nc.tensor.matmul(
                                    ps[:, :], st["uT"][:, us, ds_],
                                    exps[s][:, us, halves[h]],
                                    start=(u == 0), stop=(u == NT // 2 - 1),
                                    perf_mode=DR,
                                )
                            nc.vector.tensor_mul(fins[s][:, halves[h]], ps[:, :],
                                                 recips[s][:, halves[h]])
                    for s, st in enumerate(sts):
                        xf = st["x"].bitcast(F32)
                        eng(FINADD_ENG[d]).tensor_add(
                            fins[s][:, :], fins[s][:, :], xf[:, d, :])
                        nc.scalar.dma_start(
                            out=out_d[st["img"], 128 * d:128 * (d + 1), :],
                            in_=fins[s][:, :],
                        )

            if loop_iters:
                # software-pipelined + 2x unrolled: the prologue produces
                # generation 0; each unrolled half consumes one generation
                # while producing the other, so no tight write-after-read
                # cycles on the state buffers. Total invocations = loop_iters.
                UNROLL = 8   # invocations per For_i iteration (amortizes
                             # the all-engine barrier at the loop back edge)
                assert loop_iters % UNROLL == 0
                prod_load(*gens[0])
                prod_gn(*gens[0])
                prod_mu(*gens[0])
                with tc.For_i(0, loop_iters // UNROLL, 1,
                              hint_engines=(mybir.EngineType.PE,
                                            mybir.EngineType.Activation,
                                            mybir.EngineType.DVE,
                                            mybir.EngineType.Pool,
                                            mybir.EngineType.SP)):
                    for rep in range(UNROLL // 2):
                        for g in (0, 1):
                            att_fwd(*gens[g])
                            att_bwd(*gens[g])
                            prod_load(*gens[1 - g])
                            prod_gn(*gens[1 - g])
                            prod_mu(*gens[1 - g])
            else:
                prod_load(*gens[0])
                prod_gn(*gens[0])
                prod_mu(*gens[0])
                att_fwd(*gens[0])
                att_bwd(*gens[0])

    _dedup_ldweights(nc)
    nc.compile()
    return nc


def _to_f8(a):
    return np.ascontiguousarray(
        np.clip(a, -240.0, 240.0).astype(ml_dtypes.float8_e4m3)
    )


def _prep_inputs(x, gn_scale, gn_bias, qkv_w, qkv_b, proj_w, proj_b):
    f = np.float32
    x_r = np.asarray(x, dtype=f).reshape(B, C, N)
    qkv_w = np.asarray(qkv_w, dtype=f)
    qkv_b = np.asarray(qkv_b, dtype=f)
    proj_w = np.asarray(proj_w, dtype=f)
    proj_b = np.asarray(proj_b, dtype=f)
    if np.any(qkv_b[0:2 * C]):
        raise NotImplementedError(
            "fused-weights kernel assumes zero q/k biases (reference uses zeros)"
        )
    # v-bias and proj-bias fold into a constant per-channel offset added to x
    # (rows of attn sum to 1): out += Wp @ bv + bp.
    bv = qkv_b[2 * C:3 * C]
    cvec = proj_w @ bv + proj_b
    if np.any(cvec):
        x_r = x_r + cvec[None, :, None]

    def col(v):
        return np.asarray(v, f).reshape(CT, 128).T

    consts = np.concatenate([col(gn_scale), col(gn_bias)], axis=1)
    indicator = (np.arange(C)[:, None] // GS == np.arange(G)[None, :]).astype(f)
    M = qkv_w[0:C].T @ qkv_w[C:2 * C]   # s_ij = xn_i^T M xn_j
    Wpr = proj_w @ qkv_w[2 * C:3 * C]   # u = W' xn
    common = {
        "wm": _to_f8(M * WSC),          # stationary [c,o]: m = wm^T xn = M^T xn
        "wu": _to_f8(Wpr.T * WSC),      # stationary [c,o]: u = wu^T xn = W' xn
        "ind16": np.ascontiguousarray(indicator / GS),
        "bind": np.ascontiguousarray(indicator.T),
        "onesm": np.full((128, 256), 16.0, dtype=ml_dtypes.float8_e4m3),
        "consts": np.ascontiguousarray(consts),
    }
    in_maps = []
    for i in range(NCORES):
        m = dict(common)
        m["x"] = np.ascontiguousarray(x_r[BPC * i:BPC * (i + 1)])
        in_maps.append(m)
    return in_maps, True


def kernel(x, gn_scale, gn_bias, qkv_w, qkv_b, proj_w, proj_b, _trace=False):
    in_maps, _ = _prep_inputs(x, gn_scale, gn_bias, qkv_w, qkv_b,
                              proj_w, proj_b)
    if "nc" not in _cache:
        _cache["nc"] = _build()
    nc = _cache["nc"]
    res = run_bass_kernel_spmd(nc, in_maps, core_ids=list(range(NCORES)),
                               trace=_trace)
    _cache["last_result"] = res
    out = np.stack([r["out"] for r in res.results], axis=0)
    return out.reshape(B, C, H, W)


# revision 39
# speedup vs baseline: 1.3824x; 1.0552x over previous
"""AttentionBlock (GroupNorm(32) + 1-head self-attention + proj + residual) on 8 trn2 cores.

Data-parallel over batch: each of the 8 NeuronCores processes 2 of the 16 images.

Algebraic fusion (valid because the reference's q/k biases are zero):
  scores: s_ij = q_i.k_j = xn_i^T (Wq^T Wk) xn_j. Precompute M = Wq^T Wk
          host-side, compute m = M^T xn on device (ONE projection instead of
          q and k), then s_ij = m_i . xn_j with xn as the stationary operand.
  output: proj(attn@V) = sum_j p_j (Wp Wv xn_j). Precompute W' = Wp Wv, so
          u = W' xn replaces v and the separate projection matmul vanishes.

All large matmuls run in fp8e4 with perf_mode=DoubleRow (K=256 per pass).
Scaling scheme: M and W' are scaled by 16 host-side; the softmax scale c^-0.5
and the 1/16 fold into the exp activation's scale; a -2.0 bias keeps exp
outputs <= ~40 (cancels in softmax). u carries 16x, which cancels against the
all-16s matmul used for the softmax denominator (recip of 16*l).

Engine balancing (v2): GroupNorm statistics via DVE bn_stats/bn_aggr (one
pass over x, no PSUM stats matmuls, no x^2 materialization); per-channel
(mean, E[x^2]) reduced to groups by tiny f32r indicator matmuls. PSUM tiles
are [128,1024] (2 banks) so PSUM->SBUF evacuations and exp run as single
1024-wide instructions. Evacuations and the fin tail are split across
ACT/DVE/Pool (gpsimd) per the engine-assignment tables below; Pool was idle
in the baseline. Emission interleaves the two images so ACT (exp+xn, the
busiest engine) stays fed: img b's bn_stats run under img a's attention, and
img a's attn@U runs while ACT converts xn_b.
"""

import ml_dtypes
import numpy as np

import concourse.bacc as bacc
import concourse.tile as tile
import concourse.mybir as mybir
from concourse.bass_utils import run_bass_kernel_spmd

F32 = mybir.dt.float32
F32R = mybir.dt.float32r
F8 = mybir.dt.float8e4
I32 = mybir.dt.int32
AF = mybir.ActivationFunctionType
ALU = mybir.AluOpType
AX = mybir.AxisListType
DR = mybir.MatmulPerfMode.DoubleRow

B, C, H, W = 16, 512, 32, 32
N = H * W                 # 1024 positions
NCORES = 8
BPC = B // NCORES         # 2 images per core
G = 32                    # groupnorm groups
GS = C // G               # 16 channels per group
CT = C // 128             # 4 channel tiles
NT = N // 128             # 8 position tiles
EPS = 1e-5
SCALE = float(C) ** -0.5  # single head, head_dim = C
WSC = 16.0                # host-side weight scale (power of 2, exact in fp8)
EXP_SCALE = SCALE / WSC   # m carries 16x; exp undoes it + softmax scale
EXP_BIAS = -2.0
MAGIC = 0x5F3759DF        # Newton-rsqrt seed constant

# engine assignment tables (tuned against TimelineSim). GPSIMD (Pool) cannot
# touch PSUM, so PSUM evacuations split across ACT/DVE and Pool gets the
# SBUF-only work (xn quantize, residual adds).
UT_COPY_ENG = ["act", "act", "act", "dve"]      # per n-pair
FINADD_ENG = ["pool", "pool", "pool", "dve"]    # per d-tile
XN_ENG = "pool"
BODY_ORDER = "v1"   # v1: ATT(a),PROD(a),ATT(b),PROD(b); v2: ATT,ATT,PROD,PROD

_cache: dict = {}


def _dedup_ldweights(nc):
    """Drop InstLdweights that reload the identical weights AP."""
    ndrop = 0
    for f in nc.m.functions:
        for blk in f.blocks:
            insts = list(blk.instructions)
            drop = []
            last_key = None
            for idx, inst in enumerate(insts):
                nm = type(inst).__name__
                if nm == "InstLdweights":
                    si = inst.sync_info
                    has_sync = si is not None and (
                        len(si.on_wait) > 0 or len(si.on_update) > 0
                    )
                    key = str(inst.ins[0])
                    if key == last_key and not has_sync:
                        drop.append(idx)
                    else:
                        last_key = key
                elif nm == "InstMatmult":
                    if inst.perf_mode is None:
                        last_key = None
            for idx in reversed(drop):
                del blk.instructions[idx]
            ndrop += len(drop)
    return ndrop


def _build(loop_iters: int = 0):
    nc = bacc.Bacc("TRN2", target_bir_lowering=False, num_devices=NCORES)

    x_d = nc.dram_tensor("x", [BPC, C, N], F32R, kind="ExternalInput")
    wm_d = nc.dram_tensor("wm", [C, C], F8, kind="ExternalInput")   # 16*(Wq^T Wk)
    wu_d = nc.dram_tensor("wu", [C, C], F8, kind="ExternalInput")   # 16*(Wp Wv)^T
    ind_d = nc.dram_tensor("ind16", [C, G], F32R, kind="ExternalInput")  # 1/16 iff c//16==g
    bind_d = nc.dram_tensor("bind", [G, C], F32R, kind="ExternalInput")  # 0/1 indicator.T
    ones_d = nc.dram_tensor("onesm", [128, 256], F8, kind="ExternalInput")  # all 16.0
    consts_d = nc.dram_tensor("consts", [128, 2 * CT], F32, kind="ExternalInput")
    out_d = nc.dram_tensor("out", [BPC, C, N], F32, kind="ExternalOutput")

    halves = [slice(0, 512), slice(512, 1024)]

    with tile.TileContext(nc) as tc:
        with (
            tc.tile_pool(name="wpool", bufs=1) as wp_,
            tc.tile_pool(name="state", bufs=1) as stp,
            tc.tile_pool(name="epool", bufs=2) as epool,
            tc.tile_pool(name="fpool", bufs=2) as fpool,
            tc.tile_pool(name="rpool", bufs=2) as rpool,
            tc.tile_pool(name="spool", bufs=2) as spool,
            tc.tile_pool(name="ps2", bufs=2, space="PSUM") as ps2,
            tc.tile_pool(name="psA", bufs=3, space="PSUM") as psA,
            tc.tile_pool(name="psT", bufs=1, space="PSUM") as psT,
        ):
            # ---- persistent constants / weights (batched single DMAs) ----
            wm_all = wp_.tile([128, CT, C], F8, tag="wm", name="wm")
            wu_all = wp_.tile([128, CT, C], F8, tag="wu", name="wu")
            ind_all = wp_.tile([128, CT, G], F32R, tag="ind", name="ind")
            bind_all = wp_.tile([G, CT, 128], F32R, tag="bind", name="bind")
            ones_sb = wp_.tile([128, 2, 128], F8, tag="ones", name="ones")
            consts_sb = wp_.tile([128, 2 * CT], F32, tag="consts", name="consts")
            magic_sb = wp_.tile([128, 1], I32, tag="magic", name="magic")
            nc.vector.memset(magic_sb, MAGIC)
            exp_sc = wp_.tile([128, 1], F32, tag="expsc", name="expsc")
            nc.vector.memset(exp_sc, EXP_SCALE)
            exp_bi = wp_.tile([128, 1], F32, tag="expbi", name="expbi")
            nc.vector.memset(exp_bi, EXP_BIAS)
            gnsc_sb = consts_sb[:, 0 * CT:1 * CT]
            gnbi_sb = consts_sb[:, 1 * CT:2 * CT]

            def part(dram2d):
                return dram2d.rearrange("(t p) f -> p t f", p=128)

            nc.sync.dma_start(out=ind_all, in_=part(ind_d[:, :]))
            nc.sync.dma_start(out=consts_sb, in_=consts_d[:, :])
            nc.sync.dma_start(
                out=bind_all, in_=bind_d.rearrange("g (t p) -> g t p", p=128)
            )
            nc.sync.dma_start(out=wm_all, in_=part(wm_d[:, :]))
            nc.sync.dma_start(out=wu_all, in_=part(wu_d[:, :]))
            nc.sync.dma_start(
                out=ones_sb, in_=ones_d.rearrange("p (s f) -> p s f", s=2)
            )

            # per-(image, generation) persistent state: PROD writes one
            # generation while ATT consumes the other; the For_i body is
            # unrolled 2x so the generations alternate with static buffers.
            def mkstate(s):
                return {
                    "x": stp.tile([128, CT, N], F32R, tag=f"x{s}", name=f"x{s}"),
                    "xn": stp.tile([128, CT, N], F8, tag=f"xn{s}", name=f"xn{s}"),
                    "m": stp.tile([128, CT, N], F8, tag=f"m{s}", name=f"m{s}"),
                    "uT": stp.tile([128, NT, C], F8, tag=f"u{s}", name=f"u{s}"),
                }
            gens = [[mkstate("a0"), mkstate("b0")],
                    [mkstate("a1"), mkstate("b1")]]
            for g in gens:
                for i, st in enumerate(g):
                    st["img"] = i
            states = gens[0]

            def eng(name):
                return {"act": nc.scalar, "dve": nc.vector, "pool": nc.gpsimd}[name]

            def emit_gn(st):
                """bn_stats -> group agg -> a/b coefficients for one image.
                Small chain; caller interleaves the two images."""
                x_all = st["x"]
                xf = x_all.bitcast(F32)
                bst = spool.tile([128, 2 * CT, 6], F32, tag="bst", name="bst")
                for t in range(CT):
                    for h in range(2):
                        nc.vector.bn_stats(
                            out=bst[:, 2 * t + h, :], in_=xf[:, t, halves[h]]
                        )
                mv = spool.tile([128, CT, 2], F32, tag="mv", name="mv")
                for t in range(CT):
                    nc.vector.bn_aggr(
                        out=mv[:, t, :], in_=bst[:, 2 * t:2 * t + 2, :]
                    )
                msq = spool.tile([128, CT], F32, tag="msq", name="msq")
                nc.vector.tensor_mul(msq[:, :], mv[:, :, 0], mv[:, :, 0])
                m12 = spool.tile([128, CT, 2], F32R, tag="m12", name="m12")
                nc.vector.tensor_copy(m12[:, :, 0], mv[:, :, 0])
                nc.vector.tensor_add(m12[:, :, 1], mv[:, :, 1], msq[:, :])
                tiny = psT.tile([128, 16], F32, tag="tiny", name="tiny")
                gps = tiny[0:G, 0:2]
                for t in range(CT):
                    nc.tensor.matmul(
                        gps[:, :], ind_all[:, t, :], m12[:, t, :],
                        start=(t == 0), stop=(t == CT - 1),
                    )
                meang = spool.tile([G, 1], F32, tag="meang", name="meang")
                nc.vector.tensor_copy(meang[:, :], gps[:, 0:1])
                msqg = spool.tile([G, 1], F32, tag="msqg", name="msqg")
                vpe = spool.tile([G, 1], F32, tag="vpe", name="vpe")
                nc.vector.tensor_mul(msqg[:, :], meang[:, :], meang[:, :])
                nc.vector.scalar_tensor_tensor(
                    out=vpe[:, :], in0=gps[:, 1:2], scalar=EPS, in1=msqg[:, :],
                    op0=ALU.add, op1=ALU.subtract,
                )
                sh_t = spool.tile([G, 1], I32, tag="sh", name="sh")
                nc.vector.tensor_scalar(
                    out=sh_t[:, :], in0=vpe.bitcast(I32)[:, :], scalar1=1,
                    scalar2=None, op0=ALU.logical_shift_right,
                )
                seed = spool.tile([G, 1], I32, tag="seed", name="seed")
                nc.vector.scalar_tensor_tensor(
                    out=seed[:, :], in0=magic_sb[:G, :], scalar=0, in1=sh_t[:, :],
                    op0=ALU.bypass, op1=ALU.subtract,
                )
                y = seed.bitcast(F32)
                for it in range(2):
                    t1 = spool.tile([G, 1], F32, tag=f"nr{it}", name=f"nr{it}")
                    nc.vector.tensor_mul(t1[:, :], y[:, :], y[:, :])
                    nc.vector.tensor_mul(t1[:, :], t1[:, :], vpe[:, :])
                    nc.vector.tensor_scalar(
                        out=t1[:, :], in0=t1[:, :], scalar1=-0.5, scalar2=1.5,
                        op0=ALU.mult, op1=ALU.add,
                    )
                    y2 = spool.tile([G, 1], F32, tag=f"y{it}", name=f"y{it}")
                    nc.vector.tensor_mul(y2[:, :], y[:, :], t1[:, :])
                    y = y2
                stats2 = spool.tile([G, 2], F32R, tag="st2", name="st2")
                nc.vector.tensor_copy(stats2[:, 0:1], y[:, :])
                nc.vector.tensor_copy(stats2[:, 1:2], meang[:, :])
                bc_ps = tiny[:, 8:8 + 2 * CT]
                for t in range(CT):
                    nc.tensor.matmul(
                        bc_ps[:, 2 * t:2 * t + 2], bind_all[:, t, :], stats2[:, :],
                        start=True, stop=True,
                    )
                bc_v = bc_ps.rearrange("p (t s) -> p t s", s=2)
                a_all = spool.tile([128, CT], F32, tag="aall", name="aall")
                b_all = spool.tile([128, CT], F32, tag="ball", name="ball")
                nc.vector.tensor_mul(a_all[:, :], gnsc_sb, bc_v[:, :, 0])
                nc.vector.scalar_tensor_tensor(
                    out=b_all[:, :], in0=bc_v[:, :, 1], scalar=-1.0, in1=a_all[:, :],
                    op0=ALU.mult, op1=ALU.mult,
                )
                nc.vector.tensor_add(b_all[:, :], b_all[:, :], gnbi_sb)
                return a_all, b_all

            def prod_load(st0, st1):
                """x loads for both images, emitted at body start so the SP
                queue issues them before the fin-gated out DMAs."""
                for st in (st0, st1):
                    x_all = st["x"]
                    for t in range(CT):
                        for h in range(2):
                            nc.sync.dma_start(
                                out=x_all[:, t, halves[h]],
                                in_=x_d[st["img"], 128 * t:128 * (t + 1),
                                        halves[h]],
                            )

            def prod_gn(st0, st1):
                sts = [st0, st1]
                abs_ = [emit_gn(st0), emit_gn(st1)]
                # tiles 0-1 on DVE (immediately after the stats chain in
                # DVE's queue, unblocking the m/u projections' first pass);
                # tiles 2-3 on Pool in parallel
                for t in range(CT):
                    for s, st in enumerate(sts):
                        a_all, b_all = abs_[s]
                        e = "dve" if t < 2 else XN_ENG
                        eng(e).tensor_scalar(
                            out=st["xn"][:, t, :],
                            in0=st["x"].bitcast(F32)[:, t, :],
                            scalar1=a_all[:, t:t + 1],
                            scalar2=b_all[:, t:t + 1],
                            op0=ALU.mult, op1=ALU.add,
                        )

            def prod_mu(st0, st1):
                sts = [st0, st1]
                for d in range(CT):
                    ds_ = slice(128 * d, 128 * (d + 1))
                    for st in sts:
                        ps = ps2.tile([128, N], F32, tag="ps2", name="ps2")
                        for u in range(CT // 2):
                            us = slice(2 * u, 2 * u + 2)
                            for h in range(2):
                                nc.tensor.matmul(
                                    ps[:, halves[h]], wm_all[:, us, ds_],
                                    st["xn"][:, us, halves[h]],
                                    start=(u == 0), stop=(u == CT // 2 - 1),
                                    perf_mode=DR,
                                )
                        if d >= CT - 2:
                            nc.vector.tensor_copy(st["m"][:, d, :], ps[:, :])
                        else:
                            nc.scalar.copy(out=st["m"][:, d, :], in_=ps[:, :])
                for np_ in range(NT // 2):
                    for st in sts:
                        ps = ps2.tile([128, N], F32, tag="ps2", name="ps2")
                        psv = ps.rearrange("p (k f) -> p k f", k=2)
                        for k in range(2):
                            n = 2 * np_ + k
                            ns = slice(128 * n, 128 * (n + 1))
                            for u in range(CT // 2):
                                us = slice(2 * u, 2 * u + 2)
                                nc.tensor.matmul(
                                    psv[:, k, :], st["xn"][:, us, ns],
                                    wu_all[:, us, :],
                                    start=(u == 0), stop=(u == CT // 2 - 1),
                                    perf_mode=DR,
                                )
                        e = UT_COPY_ENG[np_]
                        if e == "act":
                            nc.scalar.copy(
                                out=st["uT"][:, 2 * np_:2 * np_ + 2, :],
                                in_=psv[:, :, :],
                            )
                        else:
                            eng(e).tensor_copy(
                                st["uT"][:, 2 * np_:2 * np_ + 2, :], psv[:, :, :]
                            )

            att_ctx = {}

            def att_fwd(st0, st1):
                """S/exp/L/recip for both images, tile-interleaved."""
                sts = [st0, st1]
                exps, recips = [], []
                for st in sts:
                    exps.append(epool.tile([128, NT, N], F8, tag="e", name="e"))
                    recips.append(rpool.tile([128, N], F32, tag="rbc", name="rbc"))
                for j in range(NT):
                    js = slice(128 * j, 128 * (j + 1))
                    for s, st in enumerate(sts):
                        ps = ps2.tile([128, N], F32, tag="ps2", name="ps2")
                        for u in range(CT // 2):
                            us = slice(2 * u, 2 * u + 2)
                            for h in range(2):
                                nc.tensor.matmul(
                                    ps[:, halves[h]], st["xn"][:, us, js],
                                    st["m"][:, us, halves[h]],
                                    start=(u == 0), stop=(u == CT // 2 - 1),
                                    perf_mode=DR,
                                )
                        nc.scalar.activation(out=exps[s][:, j, :], in_=ps[:, :],
                                             func=AF.Exp, scale=exp_sc[:, 0:1],
                                             bias=exp_bi[:, 0:1])
                lps = []
                for s in range(2):
                    ps = ps2.tile([128, N], F32, tag="ps2", name="ps2")
                    for u in range(NT // 2):
                        us = slice(2 * u, 2 * u + 2)
                        for h in range(2):
                            nc.tensor.matmul(
                                ps[:, halves[h]], ones_sb[:, :, :],
                                exps[s][:, us, halves[h]],
                                start=(u == 0), stop=(u == NT // 2 - 1),
                                perf_mode=DR,
                            )
                    lps.append(ps)
                for s in range(2):
                    nc.vector.reciprocal_approx_fast(out=recips[s][:, :],
                                                     in_=lps[s][:, :])
                att_ctx["exps"], att_ctx["recips"] = exps, recips

            def att_bwd(st0, st1):
                """attn@U, normalize, residual, writeback for both images."""
                sts = [st0, st1]
                exps, recips = att_ctx["exps"], att_ctx["recips"]
                for d in range(CT):
                    ds_ = slice(128 * d, 128 * (d + 1))
                    fins = []
                    for s, st in enumerate(sts):
                        fin = fpool.tile([128, N], F32, tag="fin", name="fin")
                        fins.append(fin)
                        for h in range(2):
                            ps = psA.tile([128, 512], F32, tag="psA", name="psA")
                            for u in range(NT // 2):
                                us = slice(2 * u, 2 * u + 2)
                                nc.tensor.matmul(
                                    ps[:, :], st["uT"][:, us, ds_],
                                    exps[s][:, us, halves[h]],
                                    start=(u == 0), stop=(u == NT // 2 - 1),
                                    perf_mode=DR,
                                )
                            nc.vector.tensor_mul(fins[s][:, halves[h]], ps[:, :],
                                                 recips[s][:, halves[h]])
                    for s, st in enumerate(sts):
                        xf = st["x"].bitcast(F32)
                        eng(FINADD_ENG[d]).tensor_add(
                            fins[s][:, :], fins[s][:, :], xf[:, d, :])
                        nc.scalar.dma_start(
                            out=out_d[st["img"], 128 * d:128 * (d + 1), :],
                            in_=fins[s][:, :],
                        )

            if loop_iters:
                # software-pipelined + 2x unrolled: the prologue produces
                # generation 0; each unrolled half consumes one generation
                # while producing the other, so no tight write-after-read
                # cycles on the state buffers. Total invocations = loop_iters.
                UNROLL = 8   # invocations per For_i iteration (amortizes
                             # the all-engine barrier at the loop back edge)
                assert loop_iters % UNROLL == 0
                prod_load(*gens[0])
                prod_gn(*gens[0])
                prod_mu(*gens[0])
                with tc.For_i(0, loop_iters // UNROLL, 1,
                              hint_engines=(mybir.EngineType.PE,
                                            mybir.EngineType.Activation,
                                            mybir.EngineType.DVE,
                                            mybir.EngineType.Pool,
                                            mybir.EngineType.SP)):
                    for rep in range(UNROLL // 2):
                        for g in (0, 1):
                            att_fwd(*gens[g])
                            att_bwd(*gens[g])
                            prod_load(*gens[1 - g])
                            prod_gn(*gens[1 - g])
                            prod_mu(*gens[1 - g])
            else:
                prod_load(*gens[0])
                prod_gn(*gens[0])
                prod_mu(*gens[0])
                att_fwd(*gens[0])
                att_bwd(*gens[0])

    _dedup_ldweights(nc)
    nc.compile()
    return nc


def _to_f8(a):
    return np.ascontiguousarray(
        np.clip(a, -240.0, 240.0).astype(ml_dtypes.float8_e4m3)
    )


def _prep_inputs(x, gn_scale, gn_bias, qkv_w, qkv_b, proj_w, proj_b):
    f = np.float32
    x_r = np.asarray(x, dtype=f).reshape(B, C, N)
    qkv_w = np.asarray(qkv_w, dtype=f)
    qkv_b = np.asarray(qkv_b, dtype=f)
    proj_w = np.asarray(proj_w, dtype=f)
    proj_b = np.asarray(proj_b, dtype=f)
    if np.any(qkv_b[0:2 * C]):
        raise NotImplementedError(
            "fused-weights kernel assumes zero q/k biases (reference uses zeros)"
        )
    # v-bias and proj-bias fold into a constant per-channel offset added to x
    # (rows of attn sum to 1): out += Wp @ bv + bp.
    bv = qkv_b[2 * C:3 * C]
    cvec = proj_w @ bv + proj_b
    if np.any(cvec):
        x_r = x_r + cvec[None, :, None]

    def col(v):
        return np.asarray(v, f).reshape(CT, 128).T

    consts = np.concatenate([col(gn_scale), col(gn_bias)], axis=1)
    indicator = (np.arange(C)[:, None] // GS == np.arange(G)[None, :]).astype(f)
    M = qkv_w[0:C].T @ qkv_w[C:2 * C]   # s_ij = xn_i^T M xn_j
    Wpr = proj_w @ qkv_w[2 * C:3 * C]   # u = W' xn
    common = {
        "wm": _to_f8(M * WSC),          # stationary [c,o]: m = wm^T xn = M^T xn
        "wu": _to_f8(Wpr.T * WSC),      # stationary [c,o]: u = wu^T xn = W' xn
        "ind16": np.ascontiguousarray(indicator / GS),
        "bind": np.ascontiguousarray(indicator.T),
        "onesm": np.full((128, 256), 16.0, dtype=ml_dtypes.float8_e4m3),
        "consts": np.ascontiguousarray(consts),
    }
    in_maps = []
    for i in range(NCORES):
        m = dict(common)
        m["x"] = np.ascontiguousarray(x_r[BPC * i:BPC * (i + 1)])
        in_maps.append(m)
    return in_maps, True


def kernel(x, gn_scale, gn_bias, qkv_w, qkv_b, proj_w, proj_b, _trace=False):
    in_maps, _ = _prep_inputs(x, gn_scale, gn_bias, qkv_w, qkv_b,
                              proj_w, proj_b)
    if "nc" not in _cache:
        _cache["nc"] = _build()
    nc = _cache["nc"]
    res = run_bass_kernel_spmd(nc, in_maps, core_ids=list(range(NCORES)),
                               trace=_trace)
    _cache["last_result"] = res
    out = np.stack([r["out"] for r in res.results], axis=0)
    return out.reshape(B, C, H, W)


# revision 44
# speedup vs baseline: 1.4525x; 1.0507x over previous
"""AttentionBlock (GroupNorm(32) + 1-head self-attention + proj + residual) on 8 trn2 cores.

Data-parallel over batch: each of the 8 NeuronCores processes 2 of the 16 images.

Algebraic fusion (valid because the reference's q/k biases are zero):
  scores: s_ij = q_i.k_j = xn_i^T (Wq^T Wk) xn_j. Precompute M = Wq^T Wk
          host-side, compute m = M^T xn on device (ONE projection instead of
          q and k), then s_ij = m_i . xn_j with xn as the stationary operand.
  output: proj(attn@V) = sum_j p_j (Wp Wv xn_j). Precompute W' = Wp Wv, so
          u = W' xn replaces v and the separate projection matmul vanishes.

All large matmuls run in fp8e4 with perf_mode=DoubleRow (K=256 per pass).
Scaling scheme: M and W' are scaled by 16 host-side; the softmax scale c^-0.5
and the 1/16 fold into the exp activation's scale; a -2.0 bias keeps exp
outputs <= ~40 (cancels in softmax). u carries 16x, which cancels against the
all-16s matmul used for the softmax denominator (recip of 16*l).

Engine balancing (v2): GroupNorm statistics via DVE bn_stats/bn_aggr (one
pass over x, no PSUM stats matmuls, no x^2 materialization); per-channel
(mean, E[x^2]) reduced to groups by tiny f32r indicator matmuls. PSUM tiles
are [128,1024] (2 banks) so PSUM->SBUF evacuations and exp run as single
1024-wide instructions. Evacuations and the fin tail are split across
ACT/DVE/Pool (gpsimd) per the engine-assignment tables below; Pool was idle
in the baseline. Emission interleaves the two images so ACT (exp+xn, the
busiest engine) stays fed: img b's bn_stats run under img a's attention, and
img a's attn@U runs while ACT converts xn_b.
"""

import ml_dtypes
import numpy as np

import concourse.bacc as bacc
import concourse.tile as tile
import concourse.mybir as mybir
from concourse.bass_utils import run_bass_kernel_spmd

F32 = mybir.dt.float32
F32R = mybir.dt.float32r
F8 = mybir.dt.float8e4
I32 = mybir.dt.int32
AF = mybir.ActivationFunctionType
ALU = mybir.AluOpType
AX = mybir.AxisListType
DR = mybir.MatmulPerfMode.DoubleRow

B, C, H, W = 16, 512, 32, 32
N = H * W                 # 1024 positions
NCORES = 8
BPC = B // NCORES         # 2 images per core
G = 32                    # groupnorm groups
GS = C // G               # 16 channels per group
CT = C // 128             # 4 channel tiles
NT = N // 128             # 8 position tiles
EPS = 1e-5
SCALE = float(C) ** -0.5  # single head, head_dim = C
WSC = 16.0                # host-side weight scale (power of 2, exact in fp8)
EXP_SCALE = SCALE / WSC   # m carries 16x; exp undoes it + softmax scale
EXP_BIAS = -2.0
MAGIC = 0x5F3759DF        # Newton-rsqrt seed constant

# engine assignment tables (tuned against TimelineSim). GPSIMD (Pool) cannot
# touch PSUM, so PSUM evacuations split across ACT/DVE and Pool gets the
# SBUF-only work (xn quantize, residual adds).
UT_COPY_ENG = ["act", "act", "act", "dve"]      # per n-pair
FINADD_ENG = ["pool", "pool", "pool", "dve"]    # per d-tile
XN_ENG = "pool"
BODY_ORDER = "v1"   # v1: ATT(a),PROD(a),ATT(b),PROD(b); v2: ATT,ATT,PROD,PROD

_cache: dict = {}


def _dedup_ldweights(nc):
    """Drop InstLdweights that reload the identical weights AP."""
    ndrop = 0
    for f in nc.m.functions:
        for blk in f.blocks:
            insts = list(blk.instructions)
            drop = []
            last_key = None
            for idx, inst in enumerate(insts):
                nm = type(inst).__name__
                if nm == "InstLdweights":
                    si = inst.sync_info
                    has_sync = si is not None and (
                        len(si.on_wait) > 0 or len(si.on_update) > 0
                    )
                    key = str(inst.ins[0])
                    if key == last_key and not has_sync:
                        drop.append(idx)
                    else:
                        last_key = key
                elif nm == "InstMatmult":
                    if inst.perf_mode is None:
                        last_key = None
            for idx in reversed(drop):
                del blk.instructions[idx]
            ndrop += len(drop)
    return ndrop


def _build(loop_iters: int = 0):
    nc = bacc.Bacc("TRN2", target_bir_lowering=False, num_devices=NCORES)

    x_d = nc.dram_tensor("x", [BPC, C, N], F32R, kind="ExternalInput")
    wm_d = nc.dram_tensor("wm", [C, C], F8, kind="ExternalInput")   # 16*(Wq^T Wk)
    wu_d = nc.dram_tensor("wu", [C, C], F8, kind="ExternalInput")   # 16*(Wp Wv)^T
    ind_d = nc.dram_tensor("ind16", [C, G], F32R, kind="ExternalInput")  # 1/16 iff c//16==g
    bind_d = nc.dram_tensor("bind", [G, C], F32R, kind="ExternalInput")  # 0/1 indicator.T
    ones_d = nc.dram_tensor("onesm", [128, 256], F8, kind="ExternalInput")  # all 16.0
    consts_d = nc.dram_tensor("consts", [128, 2 * CT], F32, kind="ExternalInput")
    out_d = nc.dram_tensor("out", [BPC, C, N], F32, kind="ExternalOutput")

    halves = [slice(0, 512), slice(512, 1024)]

    with tile.TileContext(nc) as tc:
        with (
            tc.tile_pool(name="wpool", bufs=1) as wp_,
            tc.tile_pool(name="state", bufs=1) as stp,
            tc.tile_pool(name="epool", bufs=2) as epool,
            tc.tile_pool(name="fpool", bufs=4) as fpool,
            tc.tile_pool(name="rpool", bufs=2) as rpool,
            tc.tile_pool(name="spool", bufs=2) as spool,
            tc.tile_pool(name="ps2", bufs=2, space="PSUM") as ps2,
            tc.tile_pool(name="psA", bufs=3, space="PSUM") as psA,
            tc.tile_pool(name="psT", bufs=1, space="PSUM") as psT,
        ):
            # ---- persistent constants / weights (batched single DMAs) ----
            wm_all = wp_.tile([128, CT, C], F8, tag="wm", name="wm")
            wu_all = wp_.tile([128, CT, C], F8, tag="wu", name="wu")
            ind_all = wp_.tile([128, CT, G], F32R, tag="ind", name="ind")
            bind_all = wp_.tile([G, CT, 128], F32R, tag="bind", name="bind")
            ones_sb = wp_.tile([128, 2, 128], F8, tag="ones", name="ones")
            consts_sb = wp_.tile([128, 2 * CT], F32, tag="consts", name="consts")
            magic_sb = wp_.tile([128, 1], I32, tag="magic", name="magic")
            nc.vector.memset(magic_sb, MAGIC)
            exp_sc = wp_.tile([128, 1], F32, tag="expsc", name="expsc")
            nc.vector.memset(exp_sc, EXP_SCALE)
            exp_bi = wp_.tile([128, 1], F32, tag="expbi", name="expbi")
            nc.vector.memset(exp_bi, EXP_BIAS)
            gnsc_sb = consts_sb[:, 0 * CT:1 * CT]
            gnbi_sb = consts_sb[:, 1 * CT:2 * CT]

            def part(dram2d):
                return dram2d.rearrange("(t p) f -> p t f", p=128)

            nc.sync.dma_start(out=ind_all, in_=part(ind_d[:, :]))
            nc.sync.dma_start(out=consts_sb, in_=consts_d[:, :])
            nc.sync.dma_start(
                out=bind_all, in_=bind_d.rearrange("g (t p) -> g t p", p=128)
            )
            nc.sync.dma_start(out=wm_all, in_=part(wm_d[:, :]))
            nc.sync.dma_start(out=wu_all, in_=part(wu_d[:, :]))
            nc.sync.dma_start(
                out=ones_sb, in_=ones_d.rearrange("p (s f) -> p s f", s=2)
            )

            # per-(image, generation) persistent state: PROD writes one
            # generation while ATT consumes the other; the For_i body is
            # unrolled 2x so the generations alternate with static buffers.
            def mkstate(s):
                return {
                    "x": stp.tile([128, CT, N], F32R, tag=f"x{s}", name=f"x{s}"),
                    "xn": stp.tile([128, CT, N], F8, tag=f"xn{s}", name=f"xn{s}"),
                    "m": stp.tile([128, CT, N], F8, tag=f"m{s}", name=f"m{s}"),
                    "uT": stp.tile([128, NT, C], F8, tag=f"u{s}", name=f"u{s}"),
                }
            gens = [[mkstate("a0"), mkstate("b0")],
                    [mkstate("a1"), mkstate("b1")]]
            for g in gens:
                for i, st in enumerate(g):
                    st["img"] = i
            states = gens[0]

            def eng(name):
                return {"act": nc.scalar, "dve": nc.vector, "pool": nc.gpsimd}[name]

            def emit_gn(st):
                """bn_stats -> group agg -> a/b coefficients for one image.
                Small chain; caller interleaves the two images."""
                x_all = st["x"]
                xf = x_all.bitcast(F32)
                bst = spool.tile([128, 2 * CT, 6], F32, tag="bst", name="bst")
                for t in range(CT):
                    for h in range(2):
                        nc.vector.bn_stats(
                            out=bst[:, 2 * t + h, :], in_=xf[:, t, halves[h]]
                        )
                mv = spool.tile([128, CT, 2], F32, tag="mv", name="mv")
                for t in range(CT):
                    nc.vector.bn_aggr(
                        out=mv[:, t, :], in_=bst[:, 2 * t:2 * t + 2, :]
                    )
                msq = spool.tile([128, CT], F32, tag="msq", name="msq")
                nc.vector.tensor_mul(msq[:, :], mv[:, :, 0], mv[:, :, 0])
                m12 = spool.tile([128, CT, 2], F32R, tag="m12", name="m12")
                nc.vector.tensor_copy(m12[:, :, 0], mv[:, :, 0])
                nc.vector.tensor_add(m12[:, :, 1], mv[:, :, 1], msq[:, :])
                tiny = psT.tile([128, 16], F32, tag="tiny", name="tiny")
                gps = tiny[0:G, 0:2]
                for t in range(CT):
                    nc.tensor.matmul(
                        gps[:, :], ind_all[:, t, :], m12[:, t, :],
                        start=(t == 0), stop=(t == CT - 1),
                    )
                meang = spool.tile([G, 1], F32, tag="meang", name="meang")
                nc.vector.tensor_copy(meang[:, :], gps[:, 0:1])
                msqg = spool.tile([G, 1], F32, tag="msqg", name="msqg")
                vpe = spool.tile([G, 1], F32, tag="vpe", name="vpe")
                nc.vector.tensor_mul(msqg[:, :], meang[:, :], meang[:, :])
                nc.vector.scalar_tensor_tensor(
                    out=vpe[:, :], in0=gps[:, 1:2], scalar=EPS, in1=msqg[:, :],
                    op0=ALU.add, op1=ALU.subtract,
                )
                sh_t = spool.tile([G, 1], I32, tag="sh", name="sh")
                nc.vector.tensor_scalar(
                    out=sh_t[:, :], in0=vpe.bitcast(I32)[:, :], scalar1=1,
                    scalar2=None, op0=ALU.logical_shift_right,
                )
                seed = spool.tile([G, 1], I32, tag="seed", name="seed")
                nc.vector.scalar_tensor_tensor(
                    out=seed[:, :], in0=magic_sb[:G, :], scalar=0, in1=sh_t[:, :],
                    op0=ALU.bypass, op1=ALU.subtract,
                )
                y = seed.bitcast(F32)
                for it in range(2):
                    t1 = spool.tile([G, 1], F32, tag=f"nr{it}", name=f"nr{it}")
                    nc.vector.tensor_mul(t1[:, :], y[:, :], y[:, :])
                    nc.vector.tensor_mul(t1[:, :], t1[:, :], vpe[:, :])
                    nc.vector.tensor_scalar(
                        out=t1[:, :], in0=t1[:, :], scalar1=-0.5, scalar2=1.5,
                        op0=ALU.mult, op1=ALU.add,
                    )
                    y2 = spool.tile([G, 1], F32, tag=f"y{it}", name=f"y{it}")
                    nc.vector.tensor_mul(y2[:, :], y[:, :], t1[:, :])
                    y = y2
                stats2 = spool.tile([G, 2], F32R, tag="st2", name="st2")
                nc.vector.tensor_copy(stats2[:, 0:1], y[:, :])
                nc.vector.tensor_copy(stats2[:, 1:2], meang[:, :])
                bc_ps = tiny[:, 8:8 + 2 * CT]
                for t in range(CT):
                    nc.tensor.matmul(
                        bc_ps[:, 2 * t:2 * t + 2], bind_all[:, t, :], stats2[:, :],
                        start=True, stop=True,
                    )
                bc_v = bc_ps.rearrange("p (t s) -> p t s", s=2)
                a_all = spool.tile([128, CT], F32, tag="aall", name="aall")
                b_all = spool.tile([128, CT], F32, tag="ball", name="ball")
                nc.vector.tensor_mul(a_all[:, :], gnsc_sb, bc_v[:, :, 0])
                nc.vector.scalar_tensor_tensor(
                    out=b_all[:, :], in0=bc_v[:, :, 1], scalar=-1.0, in1=a_all[:, :],
                    op0=ALU.mult, op1=ALU.mult,
                )
                nc.vector.tensor_add(b_all[:, :], b_all[:, :], gnbi_sb)
                return a_all, b_all

            def prod_load(st0, st1):
                """x loads for both images, emitted at body start so the SP
                queue issues them before the fin-gated out DMAs."""
                for st in (st0, st1):
                    x_all = st["x"]
                    for t in range(CT):
                        for h in range(2):
                            nc.sync.dma_start(
                                out=x_all[:, t, halves[h]],
                                in_=x_d[st["img"], 128 * t:128 * (t + 1),
                                        halves[h]],
                            )

            def prod_gn(st0, st1):
                sts = [st0, st1]
                abs_ = [emit_gn(st0), emit_gn(st1)]
                # tiles 0-1 on DVE (immediately after the stats chain in
                # DVE's queue, unblocking the m/u projections' first pass);
                # tiles 2-3 on Pool in parallel
                for t in range(CT):
                    for s, st in enumerate(sts):
                        a_all, b_all = abs_[s]
                        e = "dve" if t < 2 else XN_ENG
                        eng(e).tensor_scalar(
                            out=st["xn"][:, t, :],
                            in0=st["x"].bitcast(F32)[:, t, :],
                            scalar1=a_all[:, t:t + 1],
                            scalar2=b_all[:, t:t + 1],
                            op0=ALU.mult, op1=ALU.add,
                        )

            def prod_mu(st0, st1):
                sts = [st0, st1]
                for d in range(CT):
                    ds_ = slice(128 * d, 128 * (d + 1))
                    for st in sts:
                        ps = ps2.tile([128, N], F32, tag="ps2", name="ps2")
                        for u in range(CT // 2):
                            us = slice(2 * u, 2 * u + 2)
                            for h in range(2):
                                nc.tensor.matmul(
                                    ps[:, halves[h]], wm_all[:, us, ds_],
                                    st["xn"][:, us, halves[h]],
                                    start=(u == 0), stop=(u == CT // 2 - 1),
                                    perf_mode=DR,
                                )
                        if d >= CT - 2:
                            nc.vector.tensor_copy(st["m"][:, d, :], ps[:, :])
                        else:
                            nc.scalar.copy(out=st["m"][:, d, :], in_=ps[:, :])
                for np_ in range(NT // 2):
                    for st in sts:
                        ps = ps2.tile([128, N], F32, tag="ps2", name="ps2")
                        psv = ps.rearrange("p (k f) -> p k f", k=2)
                        for k in range(2):
                            n = 2 * np_ + k
                            ns = slice(128 * n, 128 * (n + 1))
                            for u in range(CT // 2):
                                us = slice(2 * u, 2 * u + 2)
                                nc.tensor.matmul(
                                    psv[:, k, :], st["xn"][:, us, ns],
                                    wu_all[:, us, :],
                                    start=(u == 0), stop=(u == CT // 2 - 1),
                                    perf_mode=DR,
                                )
                        e = UT_COPY_ENG[np_]
                        if e == "act":
                            nc.scalar.copy(
                                out=st["uT"][:, 2 * np_:2 * np_ + 2, :],
                                in_=psv[:, :, :],
                            )
                        else:
                            eng(e).tensor_copy(
                                st["uT"][:, 2 * np_:2 * np_ + 2, :], psv[:, :, :]
                            )

            att_ctx = {}

            def att_fwd(st0, st1):
                """S/exp/L/recip for both images, tile-interleaved."""
                sts = [st0, st1]
                exps, recips = [], []
                for st in sts:
                    exps.append(epool.tile([128, NT, N], F8, tag="e", name="e"))
                    recips.append(rpool.tile([128, N], F32, tag="rbc", name="rbc"))
                for j in range(NT):
                    js = slice(128 * j, 128 * (j + 1))
                    for s, st in enumerate(sts):
                        ps = ps2.tile([128, N], F32, tag="ps2", name="ps2")
                        for u in range(CT // 2):
                            us = slice(2 * u, 2 * u + 2)
                            for h in range(2):
                                nc.tensor.matmul(
                                    ps[:, halves[h]], st["xn"][:, us, js],
                                    st["m"][:, us, halves[h]],
                                    start=(u == 0), stop=(u == CT // 2 - 1),
                                    perf_mode=DR,
                                )
                        nc.scalar.activation(out=exps[s][:, j, :], in_=ps[:, :],
                                             func=AF.Exp, scale=exp_sc[:, 0:1],
                                             bias=exp_bi[:, 0:1])
                lps = []
                for s in range(2):
                    ps = ps2.tile([128, N], F32, tag="ps2", name="ps2")
                    for u in range(NT // 2):
                        us = slice(2 * u, 2 * u + 2)
                        for h in range(2):
                            nc.tensor.matmul(
                                ps[:, halves[h]], ones_sb[:, :, :],
                                exps[s][:, us, halves[h]],
                                start=(u == 0), stop=(u == NT // 2 - 1),
                                perf_mode=DR,
                            )
                    lps.append(ps)
                for s in range(2):
                    nc.vector.reciprocal_approx_fast(out=recips[s][:, :],
                                                     in_=lps[s][:, :])
                att_ctx["exps"], att_ctx["recips"] = exps, recips

            def att_bwd(st0, st1):
                """attn@U, normalize, residual, writeback for both images."""
                sts = [st0, st1]
                exps, recips = att_ctx["exps"], att_ctx["recips"]
                for d in range(CT):
                    ds_ = slice(128 * d, 128 * (d + 1))
                    fins = []
                    for s, st in enumerate(sts):
                        fin = fpool.tile([128, N], F32, tag="fin", name="fin")
                        fins.append(fin)
                        for h in range(2):
                            ps = psA.tile([128, 512], F32, tag="psA", name="psA")
                            for u in range(NT // 2):
                                us = slice(2 * u, 2 * u + 2)
                                nc.tensor.matmul(
                                    ps[:, :], st["uT"][:, us, ds_],
                                    exps[s][:, us, halves[h]],
                                    start=(u == 0), stop=(u == NT // 2 - 1),
                                    perf_mode=DR,
                                )
                            nc.vector.tensor_mul(fins[s][:, halves[h]], ps[:, :],
                                                 recips[s][:, halves[h]])
                    for s, st in enumerate(sts):
                        xf = st["x"].bitcast(F32)
                        eng(FINADD_ENG[d]).tensor_add(
                            fins[s][:, :], fins[s][:, :], xf[:, d, :])
                        nc.scalar.dma_start(
                            out=out_d[st["img"], 128 * d:128 * (d + 1), :],
                            in_=fins[s][:, :],
                        )

            if loop_iters:
                # software-pipelined + 2x unrolled: the prologue produces
                # generation 0; each unrolled half consumes one generation
                # while producing the other, so no tight write-after-read
                # cycles on the state buffers. Total invocations = loop_iters.
                UNROLL = 8   # invocations per For_i iteration (amortizes
                             # the all-engine barrier at the loop back edge)
                assert loop_iters % UNROLL == 0
                prod_load(*gens[0])
                prod_gn(*gens[0])
                prod_mu(*gens[0])
                with tc.For_i(0, loop_iters // UNROLL, 1,
                              hint_engines=(mybir.EngineType.PE,
                                            mybir.EngineType.Activation,
                                            mybir.EngineType.DVE,
                                            mybir.EngineType.Pool,
                                            mybir.EngineType.SP)):
                    for rep in range(UNROLL // 2):
                        for g in (0, 1):
                            att_fwd(*gens[g])
                            att_bwd(*gens[g])
                            prod_load(*gens[1 - g])
                            prod_gn(*gens[1 - g])
                            prod_mu(*gens[1 - g])
            else:
                prod_load(*gens[0])
                prod_gn(*gens[0])
                prod_mu(*gens[0])
                att_fwd(*gens[0])
                att_bwd(*gens[0])

    _dedup_ldweights(nc)
    nc.compile()
    return nc


def _to_f8(a):
    return np.ascontiguousarray(
        np.clip(a, -240.0, 240.0).astype(ml_dtypes.float8_e4m3)
    )


def _prep_inputs(x, gn_scale, gn_bias, qkv_w, qkv_b, proj_w, proj_b):
    f = np.float32
    x_r = np.asarray(x, dtype=f).reshape(B, C, N)
    qkv_w = np.asarray(qkv_w, dtype=f)
    qkv_b = np.asarray(qkv_b, dtype=f)
    proj_w = np.asarray(proj_w, dtype=f)
    proj_b = np.asarray(proj_b, dtype=f)
    if np.any(qkv_b[0:2 * C]):
        raise NotImplementedError(
            "fused-weights kernel assumes zero q/k biases (reference uses zeros)"
        )
    # v-bias and proj-bias fold into a constant per-channel offset added to x
    # (rows of attn sum to 1): out += Wp @ bv + bp.
    bv = qkv_b[2 * C:3 * C]
    cvec = proj_w @ bv + proj_b
    if np.any(cvec):
        x_r = x_r + cvec[None, :, None]

    def col(v):
        return np.asarray(v, f).reshape(CT, 128).T

    consts = np.concatenate([col(gn_scale), col(gn_bias)], axis=1)
    indicator = (np.arange(C)[:, None] // GS == np.arange(G)[None, :]).astype(f)
    M = qkv_w[0:C].T @ qkv_w[C:2 * C]   # s_ij = xn_i^T M xn_j
    Wpr = proj_w @ qkv_w[2 * C:3 * C]   # u = W' xn
    common = {
        "wm": _to_f8(M * WSC),          # stationary [c,o]: m = wm^T xn = M^T xn
        "wu": _to_f8(Wpr.T * WSC),      # stationary [c,o]: u = wu^T xn = W' xn
        "ind16": np.ascontiguousarray(indicator / GS),
        "bind": np.ascontiguousarray(indicator.T),
        "onesm": np.full((128, 256), 16.0, dtype=ml_dtypes.float8_e4m3),
        "consts": np.ascontiguousarray(consts),
    }
    in_maps = []
    for i in range(NCORES):
        m = dict(common)
        m["x"] = np.ascontiguousarray(x_r[BPC * i:BPC * (i + 1)])
        in_maps.append(m)
    return in_maps, True


def kernel(x, gn_scale, gn_bias, qkv_w, qkv_b, proj_w, proj_b, _trace=False):
    in_maps, _ = _prep_inputs(x, gn_scale, gn_bias, qkv_w, qkv_b,
                              proj_w, proj_b)
    if "nc" not in _cache:
        _cache["nc"] = _build()
    nc = _cache["nc"]
    res = run_bass_kernel_spmd(nc, in_maps, core_ids=list(range(NCORES)),
                               trace=_trace)
    _cache["last_result"] = res
    out = np.stack([r["out"] for r in res.results], axis=0)
    return out.reshape(B, C, H, W)


# revision 49
# speedup vs baseline: 1.4531x; 1.0004x over previous
"""AttentionBlock (GroupNorm(32) + 1-head self-attention + proj + residual) on 8 trn2 cores.

Data-parallel over batch: each of the 8 NeuronCores processes 2 of the 16 images.

Algebraic fusion (valid because the reference's q/k biases are zero):
  scores: s_ij = q_i.k_j = xn_i^T (Wq^T Wk) xn_j. Precompute M = Wq^T Wk
          host-side, compute m = M^T xn on device (ONE projection instead of
          q and k), then s_ij = m_i . xn_j with xn as the stationary operand.
  output: proj(attn@V) = sum_j p_j (Wp Wv xn_j). Precompute W' = Wp Wv, so
          u = W' xn replaces v and the separate projection matmul vanishes.

All large matmuls run in fp8e4 with perf_mode=DoubleRow (K=256 per pass).
Scaling scheme: M and W' are scaled by 16 host-side; the softmax scale c^-0.5
and the 1/16 fold into the exp activation's scale; a -2.0 bias keeps exp
outputs <= ~40 (cancels in softmax). u carries 16x, which cancels against the
all-16s matmul used for the softmax denominator (recip of 16*l).

Engine balancing (v2): GroupNorm statistics via DVE bn_stats/bn_aggr (one
pass over x, no PSUM stats matmuls, no x^2 materialization); per-channel
(mean, E[x^2]) reduced to groups by tiny f32r indicator matmuls. PSUM tiles
are [128,1024] (2 banks) so PSUM->SBUF evacuations and exp run as single
1024-wide instructions. Evacuations and the fin tail are split across
ACT/DVE/Pool (gpsimd) per the engine-assignment tables below; Pool was idle
in the baseline. Emission interleaves the two images so ACT (exp+xn, the
busiest engine) stays fed: img b's bn_stats run under img a's attention, and
img a's attn@U runs while ACT converts xn_b.
"""

import ml_dtypes
import numpy as np

import concourse.bacc as bacc
import concourse.tile as tile
import concourse.mybir as mybir
from concourse.bass_utils import run_bass_kernel_spmd

F32 = mybir.dt.float32
F32R = mybir.dt.float32r
F8 = mybir.dt.float8e4
I32 = mybir.dt.int32
AF = mybir.ActivationFunctionType
ALU = mybir.AluOpType
AX = mybir.AxisListType
DR = mybir.MatmulPerfMode.DoubleRow

B, C, H, W = 16, 512, 32, 32
N = H * W                 # 1024 positions
NCORES = 8
BPC = B // NCORES         # 2 images per core
G = 32                    # groupnorm groups
GS = C // G               # 16 channels per group
CT = C // 128             # 4 channel tiles
NT = N // 128             # 8 position tiles
EPS = 1e-5
SCALE = float(C) ** -0.5  # single head, head_dim = C
WSC = 16.0                # host-side weight scale (power of 2, exact in fp8)
EXP_SCALE = SCALE / WSC   # m carries 16x; exp undoes it + softmax scale
EXP_BIAS = -2.0
MAGIC = 0x5F3759DF        # Newton-rsqrt seed constant

# engine assignment tables (tuned against TimelineSim). GPSIMD (Pool) cannot
# touch PSUM, so PSUM evacuations split across ACT/DVE and Pool gets the
# SBUF-only work (xn quantize, residual adds).
UT_COPY_ENG = ["act", "act", "act", "dve"]      # per n-pair
FINADD_ENG = ["pool", "pool", "pool", "dve"]    # per d-tile
XN_ENG = "pool"
BODY_ORDER = "v1"   # v1: ATT(a),PROD(a),ATT(b),PROD(b); v2: ATT,ATT,PROD,PROD

_cache: dict = {}


def _dedup_ldweights(nc):
    """Drop InstLdweights that reload the identical weights AP."""
    ndrop = 0
    for f in nc.m.functions:
        for blk in f.blocks:
            insts = list(blk.instructions)
            drop = []
            last_key = None
            for idx, inst in enumerate(insts):
                nm = type(inst).__name__
                if nm == "InstLdweights":
                    si = inst.sync_info
                    has_sync = si is not None and (
                        len(si.on_wait) > 0 or len(si.on_update) > 0
                    )
                    key = str(inst.ins[0])
                    if key == last_key and not has_sync:
                        drop.append(idx)
                    else:
                        last_key = key
                elif nm == "InstMatmult":
                    if inst.perf_mode is None:
                        last_key = None
            for idx in reversed(drop):
                del blk.instructions[idx]
            ndrop += len(drop)
    return ndrop


def _build(loop_iters: int = 0):
    nc = bacc.Bacc("TRN2", target_bir_lowering=False, num_devices=NCORES)

    x_d = nc.dram_tensor("x", [BPC, C, N], F32R, kind="ExternalInput")
    wm_d = nc.dram_tensor("wm", [C, C], F8, kind="ExternalInput")   # 16*(Wq^T Wk)
    wu_d = nc.dram_tensor("wu", [C, C], F8, kind="ExternalInput")   # 16*(Wp Wv)^T
    ind_d = nc.dram_tensor("ind16", [C, G], F32R, kind="ExternalInput")  # 1/16 iff c//16==g
    bind_d = nc.dram_tensor("bind", [G, C], F32R, kind="ExternalInput")  # 0/1 indicator.T
    ones_d = nc.dram_tensor("onesm", [128, 256], F8, kind="ExternalInput")  # all 16.0
    consts_d = nc.dram_tensor("consts", [128, 2 * CT], F32, kind="ExternalInput")
    out_d = nc.dram_tensor("out", [BPC, C, N], F32, kind="ExternalOutput")

    halves = [slice(0, 512), slice(512, 1024)]

    with tile.TileContext(nc) as tc:
        with (
            tc.tile_pool(name="wpool", bufs=1) as wp_,
            tc.tile_pool(name="state", bufs=1) as stp,
            tc.tile_pool(name="epool", bufs=2) as epool,
            tc.tile_pool(name="fpool", bufs=4) as fpool,
            tc.tile_pool(name="rpool", bufs=2) as rpool,
            tc.tile_pool(name="spool", bufs=2) as spool,
            tc.tile_pool(name="ps2", bufs=2, space="PSUM") as ps2,
            tc.tile_pool(name="psA", bufs=3, space="PSUM") as psA,
            tc.tile_pool(name="psT", bufs=1, space="PSUM") as psT,
        ):
            # ---- persistent constants / weights (batched single DMAs) ----
            wm_all = wp_.tile([128, CT, C], F8, tag="wm", name="wm")
            wu_all = wp_.tile([128, CT, C], F8, tag="wu", name="wu")
            ind_all = wp_.tile([128, CT, G], F32R, tag="ind", name="ind")
            bind_all = wp_.tile([G, CT, 128], F32R, tag="bind", name="bind")
            ones_sb = wp_.tile([128, 2, 128], F8, tag="ones", name="ones")
            consts_sb = wp_.tile([128, 2 * CT], F32, tag="consts", name="consts")
            magic_sb = wp_.tile([128, 1], I32, tag="magic", name="magic")
            nc.vector.memset(magic_sb, MAGIC)
            exp_sc = wp_.tile([128, 1], F32, tag="expsc", name="expsc")
            nc.vector.memset(exp_sc, EXP_SCALE)
            exp_bi = wp_.tile([128, 1], F32, tag="expbi", name="expbi")
            nc.vector.memset(exp_bi, EXP_BIAS)
            gnsc_sb = consts_sb[:, 0 * CT:1 * CT]
            gnbi_sb = consts_sb[:, 1 * CT:2 * CT]

            def part(dram2d):
                return dram2d.rearrange("(t p) f -> p t f", p=128)

            nc.sync.dma_start(out=ind_all, in_=part(ind_d[:, :]))
            nc.sync.dma_start(out=consts_sb, in_=consts_d[:, :])
            nc.sync.dma_start(
                out=bind_all, in_=bind_d.rearrange("g (t p) -> g t p", p=128)
            )
            nc.sync.dma_start(out=wm_all, in_=part(wm_d[:, :]))
            nc.sync.dma_start(out=wu_all, in_=part(wu_d[:, :]))
            nc.sync.dma_start(
                out=ones_sb, in_=ones_d.rearrange("p (s f) -> p s f", s=2)
            )

            # per-(image, generation) persistent state: PROD writes one
            # generation while ATT consumes the other; the For_i body is
            # unrolled 2x so the generations alternate with static buffers.
            def mkstate(s):
                return {
                    "x": stp.tile([128, CT, N], F32R, tag=f"x{s}", name=f"x{s}"),
                    "xn": stp.tile([128, CT, N], F8, tag=f"xn{s}", name=f"xn{s}"),
                    "m": stp.tile([128, CT, N], F8, tag=f"m{s}", name=f"m{s}"),
                    "uT": stp.tile([128, NT, C], F8, tag=f"u{s}", name=f"u{s}"),
                }
            gens = [[mkstate("a0"), mkstate("b0")],
                    [mkstate("a1"), mkstate("b1")]]
            for g in gens:
                for i, st in enumerate(g):
                    st["img"] = i
            states = gens[0]

            def eng(name):
                return {"act": nc.scalar, "dve": nc.vector, "pool": nc.gpsimd}[name]

            def emit_gn(st):
                """bn_stats -> group agg -> a/b coefficients for one image.
                Small chain; caller interleaves the two images."""
                x_all = st["x"]
                xf = x_all.bitcast(F32)
                bst = spool.tile([128, 2 * CT, 6], F32, tag="bst", name="bst")
                for t in range(CT):
                    for h in range(2):
                        nc.vector.bn_stats(
                            out=bst[:, 2 * t + h, :], in_=xf[:, t, halves[h]]
                        )
                mv = spool.tile([128, CT, 2], F32, tag="mv", name="mv")
                for t in range(CT):
                    nc.vector.bn_aggr(
                        out=mv[:, t, :], in_=bst[:, 2 * t:2 * t + 2, :]
                    )
                msq = spool.tile([128, CT], F32, tag="msq", name="msq")
                nc.vector.tensor_mul(msq[:, :], mv[:, :, 0], mv[:, :, 0])
                m12 = spool.tile([128, CT, 2], F32R, tag="m12", name="m12")
                nc.vector.tensor_copy(m12[:, :, 0], mv[:, :, 0])
                nc.vector.tensor_add(m12[:, :, 1], mv[:, :, 1], msq[:, :])
                tiny = psT.tile([128, 16], F32, tag="tiny", name="tiny")
                gps = tiny[0:G, 0:2]
                for t in range(CT):
                    nc.tensor.matmul(
                        gps[:, :], ind_all[:, t, :], m12[:, t, :],
                        start=(t == 0), stop=(t == CT - 1),
                    )
                meang = spool.tile([G, 1], F32, tag="meang", name="meang")
                nc.vector.tensor_copy(meang[:, :], gps[:, 0:1])
                msqg = spool.tile([G, 1], F32, tag="msqg", name="msqg")
                vpe = spool.tile([G, 1], F32, tag="vpe", name="vpe")
                nc.vector.tensor_mul(msqg[:, :], meang[:, :], meang[:, :])
                nc.vector.scalar_tensor_tensor(
                    out=vpe[:, :], in0=gps[:, 1:2], scalar=EPS, in1=msqg[:, :],
                    op0=ALU.add, op1=ALU.subtract,
                )
                sh_t = spool.tile([G, 1], I32, tag="sh", name="sh")
                nc.vector.tensor_scalar(
                    out=sh_t[:, :], in0=vpe.bitcast(I32)[:, :], scalar1=1,
                    scalar2=None, op0=ALU.logical_shift_right,
                )
                seed = spool.tile([G, 1], I32, tag="seed", name="seed")
                nc.vector.scalar_tensor_tensor(
                    out=seed[:, :], in0=magic_sb[:G, :], scalar=0, in1=sh_t[:, :],
                    op0=ALU.bypass, op1=ALU.subtract,
                )
                y = seed.bitcast(F32)
                for it in range(2):
                    t1 = spool.tile([G, 1], F32, tag=f"nr{it}", name=f"nr{it}")
                    nc.vector.tensor_mul(t1[:, :], y[:, :], y[:, :])
                    nc.vector.tensor_mul(t1[:, :], t1[:, :], vpe[:, :])
                    nc.vector.tensor_scalar(
                        out=t1[:, :], in0=t1[:, :], scalar1=-0.5, scalar2=1.5,
                        op0=ALU.mult, op1=ALU.add,
                    )
                    y2 = spool.tile([G, 1], F32, tag=f"y{it}", name=f"y{it}")
                    nc.vector.tensor_mul(y2[:, :], y[:, :], t1[:, :])
                    y = y2
                stats2 = spool.tile([G, 2], F32R, tag="st2", name="st2")
                nc.vector.tensor_copy(stats2[:, 0:1], y[:, :])
                nc.vector.tensor_copy(stats2[:, 1:2], meang[:, :])
                bc_ps = tiny[:, 8:8 + 2 * CT]
                for t in range(CT):
                    nc.tensor.matmul(
                        bc_ps[:, 2 * t:2 * t + 2], bind_all[:, t, :], stats2[:, :],
                        start=True, stop=True,
                    )
                bc_v = bc_ps.rearrange("p (t s) -> p t s", s=2)
                a_all = spool.tile([128, CT], F32, tag="aall", name="aall")
                b_all = spool.tile([128, CT], F32, tag="ball", name="ball")
                nc.vector.tensor_mul(a_all[:, :], gnsc_sb, bc_v[:, :, 0])
                nc.vector.scalar_tensor_tensor(
                    out=b_all[:, :], in0=bc_v[:, :, 1], scalar=-1.0, in1=a_all[:, :],
                    op0=ALU.mult, op1=ALU.mult,
                )
                nc.vector.tensor_add(b_all[:, :], b_all[:, :], gnbi_sb)
                return a_all, b_all

            def prod_load(st0, st1):
                """x loads for both images, emitted at body start so the SP
                queue issues them before the fin-gated out DMAs."""
                for st in (st0, st1):
                    x_all = st["x"]
                    for t in range(CT):
                        for h in range(2):
                            nc.sync.dma_start(
                                out=x_all[:, t, halves[h]],
                                in_=x_d[st["img"], 128 * t:128 * (t + 1),
                                        halves[h]],
                            )

            def prod_gn(st0, st1):
                sts = [st0, st1]
                abs_ = [emit_gn(st0), emit_gn(st1)]
                # tiles 0-1 on DVE (immediately after the stats chain in
                # DVE's queue, unblocking the m/u projections' first pass);
                # tiles 2-3 on Pool in parallel
                for t in range(CT):
                    for s, st in enumerate(sts):
                        a_all, b_all = abs_[s]
                        e = "dve" if t < 2 else XN_ENG
                        eng(e).tensor_scalar(
                            out=st["xn"][:, t, :],
                            in0=st["x"].bitcast(F32)[:, t, :],
                            scalar1=a_all[:, t:t + 1],
                            scalar2=b_all[:, t:t + 1],
                            op0=ALU.mult, op1=ALU.add,
                        )

            def prod_mu(st0, st1):
                sts = [st0, st1]
                for d in range(CT):
                    ds_ = slice(128 * d, 128 * (d + 1))
                    for st in sts:
                        ps = ps2.tile([128, N], F32, tag="ps2", name="ps2")
                        for u in range(CT // 2):
                            us = slice(2 * u, 2 * u + 2)
                            for h in range(2):
                                nc.tensor.matmul(
                                    ps[:, halves[h]], wm_all[:, us, ds_],
                                    st["xn"][:, us, halves[h]],
                                    start=(u == 0), stop=(u == CT // 2 - 1),
                                    perf_mode=DR,
                                )
                        if d >= CT - 2:
                            nc.vector.tensor_copy(st["m"][:, d, :], ps[:, :])
                        else:
                            nc.scalar.copy(out=st["m"][:, d, :], in_=ps[:, :])
                for np_ in range(NT // 2):
                    for st in sts:
                        ps = ps2.tile([128, N], F32, tag="ps2", name="ps2")
                        psv = ps.rearrange("p (k f) -> p k f", k=2)
                        for k in range(2):
                            n = 2 * np_ + k
                            ns = slice(128 * n, 128 * (n + 1))
                            for u in range(CT // 2):
                                us = slice(2 * u, 2 * u + 2)
                                nc.tensor.matmul(
                                    psv[:, k, :], st["xn"][:, us, ns],
                                    wu_all[:, us, :],
                                    start=(u == 0), stop=(u == CT // 2 - 1),
                                    perf_mode=DR,
                                )
                        e = UT_COPY_ENG[np_]
                        if e == "act":
                            nc.scalar.copy(
                                out=st["uT"][:, 2 * np_:2 * np_ + 2, :],
                                in_=psv[:, :, :],
                            )
                        else:
                            eng(e).tensor_copy(
                                st["uT"][:, 2 * np_:2 * np_ + 2, :], psv[:, :, :]
                            )

            att_ctx = {}

            def att_fwd(st0, st1):
                """S/exp/L/recip for both images, tile-interleaved."""
                sts = [st0, st1]
                exps, recips = [], []
                for st in sts:
                    exps.append(epool.tile([128, NT, N], F8, tag="e", name="e"))
                    recips.append(rpool.tile([128, N], F32, tag="rbc", name="rbc"))
                for j in range(NT):
                    js = slice(128 * j, 128 * (j + 1))
                    for s, st in enumerate(sts):
                        ps = ps2.tile([128, N], F32, tag="ps2", name="ps2")
                        for u in range(CT // 2):
                            us = slice(2 * u, 2 * u + 2)
                            for h in range(2):
                                nc.tensor.matmul(
                                    ps[:, halves[h]], st["xn"][:, us, js],
                                    st["m"][:, us, halves[h]],
                                    start=(u == 0), stop=(u == CT // 2 - 1),
                                    perf_mode=DR,
                                )
                        nc.scalar.activation(out=exps[s][:, j, :], in_=ps[:, :],
                                             func=AF.Exp, scale=exp_sc[:, 0:1],
                                             bias=exp_bi[:, 0:1])
                lps = []
                for s in range(2):
                    ps = ps2.tile([128, N], F32, tag="ps2", name="ps2")
                    for u in range(NT // 2):
                        us = slice(2 * u, 2 * u + 2)
                        for h in range(2):
                            nc.tensor.matmul(
                                ps[:, halves[h]], ones_sb[:, :, :],
                                exps[s][:, us, halves[h]],
                                start=(u == 0), stop=(u == NT // 2 - 1),
                                perf_mode=DR,
                            )
                    lps.append(ps)
                for s in range(2):
                    nc.vector.reciprocal_approx_fast(out=recips[s][:, :],
                                                     in_=lps[s][:, :])
                att_ctx["exps"], att_ctx["recips"] = exps, recips

            def att_bwd(st0, st1):
                """attn@U, normalize, residual, writeback for both images."""
                sts = [st0, st1]
                exps, recips = att_ctx["exps"], att_ctx["recips"]
                for d in range(CT):
                    ds_ = slice(128 * d, 128 * (d + 1))
                    fins = []
                    for s, st in enumerate(sts):
                        fin = fpool.tile([128, N], F32, tag="fin", name="fin")
                        fins.append(fin)
                        for h in range(2):
                            ps = psA.tile([128, 512], F32, tag="psA", name="psA")
                            for u in range(NT // 2):
                                us = slice(2 * u, 2 * u + 2)
                                nc.tensor.matmul(
                                    ps[:, :], st["uT"][:, us, ds_],
                                    exps[s][:, us, halves[h]],
                                    start=(u == 0), stop=(u == NT // 2 - 1),
                                    perf_mode=DR,
                                )
                            nc.vector.tensor_mul(fins[s][:, halves[h]], ps[:, :],
                                                 recips[s][:, halves[h]])
                    for s, st in enumerate(sts):
                        xf = st["x"].bitcast(F32)
                        eng(FINADD_ENG[d]).tensor_add(
                            fins[s][:, :], fins[s][:, :], xf[:, d, :])
                        nc.scalar.dma_start(
                            out=out_d[st["img"], 128 * d:128 * (d + 1), :],
                            in_=fins[s][:, :],
                        )

            if loop_iters:
                # software-pipelined + 2x unrolled: the prologue produces
                # generation 0; each unrolled half consumes one generation
                # while producing the other, so no tight write-after-read
                # cycles on the state buffers. Total invocations = loop_iters.
                UNROLL = 8   # invocations per For_i iteration (amortizes
                             # the all-engine barrier at the loop back edge)
                assert loop_iters % UNROLL == 0
                prod_load(*gens[0])
                prod_gn(*gens[0])
                prod_mu(*gens[0])
                with tc.For_i(0, loop_iters // UNROLL, 1,
                              hint_engines=(mybir.EngineType.PE,
                                            mybir.EngineType.Activation,
                                            mybir.EngineType.DVE,
                                            mybir.EngineType.Pool,
                                            mybir.EngineType.SP)):
                    for rep in range(UNROLL // 2):
                        for g in (0, 1):
                            att_fwd(*gens[g])
                            prod_load(*gens[1 - g])
                            prod_gn(*gens[1 - g])
                            att_bwd(*gens[g])
                            prod_mu(*gens[1 - g])
            else:
                prod_load(*gens[0])
                prod_gn(*gens[0])
                prod_mu(*gens[0])
                att_fwd(*gens[0])
                att_bwd(*gens[0])

    _dedup_ldweights(nc)
    nc.compile()
    return nc


def _to_f8(a):
    return np.ascontiguousarray(
        np.clip(a, -240.0, 240.0).astype(ml_dtypes.float8_e4m3)
    )


def _prep_inputs(x, gn_scale, gn_bias, qkv_w, qkv_b, proj_w, proj_b):
    f = np.float32
    x_r = np.asarray(x, dtype=f).reshape(B, C, N)
    qkv_w = np.asarray(qkv_w, dtype=f)
    qkv_b = np.asarray(qkv_b, dtype=f)
    proj_w = np.asarray(proj_w, dtype=f)
    proj_b = np.asarray(proj_b, dtype=f)
    if np.any(qkv_b[0:2 * C]):
        raise NotImplementedError(
            "fused-weights kernel assumes zero q/k biases (reference uses zeros)"
        )
    # v-bias and proj-bias fold into a constant per-channel offset added to x
    # (rows of attn sum to 1): out += Wp @ bv + bp.
    bv = qkv_b[2 * C:3 * C]
    cvec = proj_w @ bv + proj_b
    if np.any(cvec):
        x_r = x_r + cvec[None, :, None]

    def col(v):
        return np.asarray(v, f).reshape(CT, 128).T

    consts = np.concatenate([col(gn_scale), col(gn_bias)], axis=1)
    indicator = (np.arange(C)[:, None] // GS == np.arange(G)[None, :]).astype(f)
    M = qkv_w[0:C].T @ qkv_w[C:2 * C]   # s_ij = xn_i^T M xn_j
    Wpr = proj_w @ qkv_w[2 * C:3 * C]   # u = W' xn
    common = {
        "wm": _to_f8(M * WSC),          # stationary [c,o]: m = wm^T xn = M^T xn
        "wu": _to_f8(Wpr.T * WSC),      # stationary [c,o]: u = wu^T xn = W' xn
        "ind16": np.ascontiguousarray(indicator / GS),
        "bind": np.ascontiguousarray(indicator.T),
        "onesm": np.full((128, 256), 16.0, dtype=ml_dtypes.float8_e4m3),
        "consts": np.ascontiguousarray(consts),
    }
    in_maps = []
    for i in range(NCORES):
        m = dict(common)
        m["x"] = np.ascontiguousarray(x_r[BPC * i:BPC * (i + 1)])
        in_maps.append(m)
    return in_maps, True


def kernel(x, gn_scale, gn_bias, qkv_w, qkv_b, proj_w, proj_b, _trace=False):
    in_maps, _ = _prep_inputs(x, gn_scale, gn_bias, qkv_w, qkv_b,
                              proj_w, proj_b)
    if "nc" not in _cache:
        _cache["nc"] = _build()
    nc = _cache["nc"]
    res = run_bass_kernel_spmd(nc, in_maps, core_ids=list(range(NCORES)),
                               trace=_trace)
    _cache["last_result"] = res
    out = np.stack([r["out"] for r in res.results], axis=0)
    return out.reshape(B, C, H, W)
